# revision 1
# baseline (speedup 1.0000x reference)
"""Trainium2 Bass kernel for nn_ChamferNormalLoss (8-core data parallel).

Sharding: pure data parallel — one batch sample per NeuronCore; the host
averages the 8 per-core |dot| sums (the only cross-core reduction).

Per-sample pipeline on each core:
  1. Brute-force NN searches (gt: 2048x8192, pred: 2048x2688-padded) as
     TensorE matmuls with a K=4 contraction that fuses the bias:
     s = 2*q.r - |r|^2, so argmax(s) == argmin squared distance.  The
     transposed [4, N] operand layouts are built on-chip with PE
     transposes (contiguous DMA loads only; column order is a known
     permutation of vertex id, unpermuted after the search with cheap
     int ops).  ScalarE evacuates distance PSUM tiles to SBUF; VectorE
     computes the row max with one 2x-mode tensor_scalar accumulate and
     extracts the argmax with max_index (first-match = jnp tie rule).
  2. Area-weighted vertex normals WITHOUT scatter support: face corner
     vertices are fetched with per-partition-row indirect DMAs (the only
     gather form the SWDGE ucode implements: one dense [128,1] offset
     column per instruction), cross products on VectorE, and the
     scatter-add n[v] += fn is factorized via v = hi*128 + lo: for each
     (face-chunk, corner), one-hot(lo) [128f,128lo] (fp16, built on
     GPSIMD) becomes matmul weights and one-hot(hi)*fn [128f,3*64]
     (fp16, VectorE two-op tensor_scalar) the moving operand; a single
     PSUM tile accumulates G[lo, c, hi] over all 384 chunk-corner pairs.
     One-hot emission is interleaved with the search chunks so VectorE
     stays busy while ScalarE evacuates.
  3. Epilogue: indirect row-gathers of the nearest gt normal (from G in
     DRAM) and nearest pred vertex; |dot| via dot/(max(|e|,eps)*
     max(|n|,eps)) == the reference's normalize-then-dot; abs-sum reduce
     per partition; partition sum via a ones-matmul into PSUM.

Numerics: fp32 throughout the searches (float32r/bf16 were measured to
flip ~2.5% of nearest-neighbor indices on HW); one-hots/fn in fp16 with
fp32 PSUM accumulation.  End-to-end relative error vs the fp32 jax
reference is ~5e-6 on hardware.
"""

import os, sys

for _p in (
    "/opt/trn_rl_repo",
    "/opt/pypackages",
    "/root/.axon_site/_ro/trn_rl_repo",
    "/root/.axon_site/_ro/pypackages",
):
    if os.path.isdir(_p) and _p not in sys.path:
        sys.path.insert(0, _p)

import numpy as np

import concourse.bass as bass
import concourse.bacc as bacc
import concourse.tile as tile
from concourse import masks, mybir

F32 = mybir.dt.float32
FP16 = mybir.dt.float16
I32 = mybir.dt.int32
U32 = mybir.dt.uint32
A = mybir.AluOpType
AF = mybir.ActivationFunctionType
AX = mybir.AxisListType

B = 8
P, PC = 2048, 16          # queries, chunks of 128
NGT, CGT = 8192, 16       # gt vertices, n-chunks of 512
VPR, VPAD, CPR = 2562, 2688, 6
NF, FCH = 16384, 128      # faces, chunks of 128
BIGC = 1.0e6              # pad coordinate; rsq pad = 3e12

EPS = 1e-12


def build_nc(debug_outs=False):
    nc = bacc.Bacc(None, target_bir_lowering=False)
    pp = nc.dram_tensor("pred_points", [P, 3], F32, kind="ExternalInput")
    pv = nc.dram_tensor("pred_vertices", [VPR, 3], F32, kind="ExternalInput")
    gv = nc.dram_tensor("gt_vertices", [NGT, 3], F32, kind="ExternalInput")
    gf = nc.dram_tensor("gt_faces32", [NF, 3], I32, kind="ExternalInput")
    g_dram = nc.dram_tensor("g_norm", [NGT * 3, 1], F32)
    out = nc.dram_tensor("loss_sum", [1], F32, kind="ExternalOutput")

    from contextlib import ExitStack

    dbg = {}
    if debug_outs:
        for nm, shape, dt in [
            ("dbg_idx_gt", [128, PC], I32), ("dbg_idx_pr", [128, PC], I32),
            ("dbg_dot", [128, PC], F32), ("dbg_ee", [128, PC], F32),
            ("dbg_nn", [128, PC], F32), ("dbg_res", [128, PC], F32),
            ("dbg_g", [128, 192], F32), ("dbg_fn", [128, FCH * 3], F32),
            ("dbg_rt", [4, NGT], F32), ("dbg_rtp", [4, VPAD], F32),
            ("dbg_qt", [4, P], F32),
        ]:
            dbg[nm] = nc.dram_tensor(nm, shape, dt, kind="ExternalOutput")
    with tile.TileContext(nc) as tc, ExitStack() as ctx:
        _body(nc, tc, ctx, pp, pv, gv, gf, g_dram, out, dbg)
    nc.compile()
    return nc


def _body(nc, tc, ctx, pp, pv, gv, gf, g_dram, out_dram, dbg=None):
    sing = ctx.enter_context(tc.tile_pool(name="sing", bufs=1))
    work = ctx.enter_context(tc.tile_pool(name="work", bufs=2))
    oh = ctx.enter_context(tc.tile_pool(name="oh", bufs=3))
    ppsum = ctx.enter_context(
        tc.tile_pool(name="ppsum", bufs=4, space=bass.MemorySpace.PSUM)
    )
    mpsum = ctx.enter_context(
        tc.tile_pool(name="mpsum", bufs=1, space=bass.MemorySpace.PSUM)
    )
    gpsum = ctx.enter_context(
        tc.tile_pool(name="gpsum", bufs=1, space=bass.MemorySpace.PSUM)
    )

    ident0 = sing.tile([128, 128], F32)
    masks.make_identity(nc, ident0[:])
    # transpose-mode matmuls can carry only one sync wait, so make the
    # identity a DVE product: every transpose then waits on DVE alone.
    ident = sing.tile([128, 128], F32)
    nc.vector.tensor_copy(ident[:], ident0[:])

    # ---- query side: qT[:, n] = [2qx, 2qy, 2qz, -1] of query (n&127)*16 + (n>>7)
    qRM = sing.tile([128, PC, 3], F32)
    nc.sync.dma_start(out=qRM[:], in_=pp[:, :].rearrange("(p i) c -> p i c", p=128))
    qCM = work.tile([128, 3, PC], F32, tag="qcm")
    nc.vector.tensor_copy(qCM[:], qRM[:].rearrange("p i c -> p c i"))
    qT = sing.tile([4, P], F32)
    nc.vector.memset(qT[:, :], -1.0)
    qtp = mpsum.tile([48, 128], F32, tag="tp")
    nc.tensor.transpose(qtp[:], qCM[:].rearrange("p c i -> p (c i)"), ident[:])
    qtsb = work.tile([48, 128], F32, tag="tsb")
    nc.vector.tensor_scalar(
        out=qtsb[:], in0=qtp[:], scalar1=2.0, scalar2=None, op0=A.mult
    )
    nc.sync.dma_start(
        out=qT[0:3, :].rearrange("c (i p) -> c i p", p=128), in_=qtsb[:]
    )

    # ---- gt side: rT[:, n] = [x, y, z, |r|^2] of vertex (n&127)*64 + (n>>7)
    rRM = work.tile([128, 64, 3], F32, tag="rrm")
    nc.sync.dma_start(out=rRM[:], in_=gv[:, :].rearrange("(p t) c -> p t c", p=128))
    rCM = work.tile([128, 2, 3, 32], F32, tag="rcm")
    nc.vector.tensor_copy(rCM[:], rRM[:].rearrange("p (h t) c -> p h c t", h=2))
    sq = work.tile([128, 64, 3], F32, tag="sq")
    nc.vector.tensor_tensor(out=sq[:], in0=rRM[:], in1=rRM[:], op=A.mult)
    rsq = work.tile([128, 64], F32, tag="rsq")
    nc.vector.tensor_reduce(out=rsq[:], in_=sq[:], axis=AX.X, op=A.add)
    rT = sing.tile([4, NGT], F32)
    for h in range(2):
        ctp = mpsum.tile([96, 128], F32, tag="tp")
        nc.tensor.transpose(
            ctp[:], rCM[:, h, :, :].rearrange("p c t -> p (c t)"), ident[:]
        )
        ctsb = work.tile([96, 128], F32, tag="tsb")
        nc.vector.tensor_copy(ctsb[:], ctp[:])
        nc.sync.dma_start(
            out=rT[0:3, h * 32 * 128 : (h + 1) * 32 * 128].rearrange(
                "c (t p) -> c t p", p=128
            ),
            in_=ctsb[:],
        )
    stp = mpsum.tile([64, 128], F32, tag="tp")
    nc.tensor.transpose(stp[:], rsq[:], ident[:])
    stsb = work.tile([64, 128], F32, tag="tsb")
    nc.vector.tensor_copy(stsb[:], stp[:])
    nc.sync.dma_start(out=rT[3:4, :], in_=stsb[:])

    # ---- pred side (padded to 2688): vertex (n&127)*21 + (n>>7)
    rRMp = work.tile([128, 21, 3], F32, tag="rrmp")
    nc.vector.memset(rRMp[:], BIGC)
    rRMp_f = rRMp[:].rearrange("p t c -> p (t c)")
    pv_f = pv[:, :].rearrange("v c -> (v c)")
    nc.sync.dma_start(
        out=rRMp_f[0:122, :],
        in_=pv_f[0 : 122 * 63].rearrange("(p a) -> p a", a=63),
    )
    rCMp = work.tile([128, 3, 21], F32, tag="rcmp")
    nc.vector.tensor_copy(rCMp[:], rRMp[:].rearrange("p t c -> p c t"))
    sqp = work.tile([128, 21, 3], F32, tag="sqp")
    nc.vector.tensor_tensor(out=sqp[:], in0=rRMp[:], in1=rRMp[:], op=A.mult)
    rsqp = work.tile([128, 21], F32, tag="rsqp")
    nc.vector.tensor_reduce(out=rsqp[:], in_=sqp[:], axis=AX.X, op=A.add)
    rTp = sing.tile([4, VPAD], F32)
    ptp = mpsum.tile([63, 128], F32, tag="tp")
    nc.tensor.transpose(ptp[:], rCMp[:].rearrange("p c t -> p (c t)"), ident[:])
    ptsb = work.tile([63, 128], F32, tag="tsb")
    nc.vector.tensor_copy(ptsb[:], ptp[:])
    nc.sync.dma_start(
        out=rTp[0:3, :].rearrange("c (t p) -> c t p", p=128), in_=ptsb[:]
    )
    sptp = mpsum.tile([21, 128], F32, tag="tp")
    nc.tensor.transpose(sptp[:], rsqp[:], ident[:])
    sptsb = work.tile([21, 128], F32, tag="tsb")
    nc.vector.tensor_copy(sptsb[:], sptp[:])
    nc.sync.dma_start(out=rTp[3:4, :], in_=sptsb[:])

    # ---------------- faces: corner indices, lo/hi decomposition ----------
    faces = sing.tile([128, FCH, 3], I32)
    nc.sync.dma_start(
        out=faces[:], in_=gf[:, :].rearrange("(p ch) w -> p ch w", p=128)
    )
    lo_i = sing.tile([128, FCH, 3], I32)
    hi_i = sing.tile([128, FCH, 3], I32)
    nc.vector.tensor_scalar(
        out=lo_i[:], in0=faces[:], scalar1=127, scalar2=None, op0=A.bitwise_and
    )
    nc.vector.tensor_scalar(
        out=hi_i[:], in0=faces[:], scalar1=7, scalar2=None, op0=A.logical_shift_right
    )
    lo_f = sing.tile([128, FCH, 3], F32)
    hi_f = sing.tile([128, FCH, 3], F32)
    nc.vector.tensor_copy(lo_f[:], lo_i[:])
    nc.vector.tensor_copy(hi_f[:], hi_i[:])

    # ---------------- iotas ----------------
    io128_i = sing.tile([128, 128], I32)
    nc.gpsimd.iota(io128_i[:], pattern=[[1, 128]], base=0, channel_multiplier=0)
    io128 = sing.tile([128, 128], FP16)
    nc.vector.tensor_copy(io128[:], io128_i[:])
    io64_i = sing.tile([128, 64], I32)
    nc.gpsimd.iota(io64_i[:], pattern=[[1, 64]], base=0, channel_multiplier=0)
    io64 = sing.tile([128, 64], FP16)
    nc.vector.tensor_copy(io64[:], io64_i[:])

    # ---------------- gather face corner vertices, cross products ---------
    faces3 = sing.tile([128, FCH, 3], I32)
    nc.vector.tensor_scalar(
        out=faces3[:], in0=faces[:], scalar1=3, scalar2=None, op0=A.mult
    )
    gv_flat = gv[:, :].rearrange("v (c one) -> (v c) one", one=1)
    Vg = sing.tile([128, FCH * 3, 3], F32)
    gcols = ctx.enter_context(tc.tile_pool(name="gcols", bufs=8))
    for j in range(FCH * 3):
        col = gcols.tile([128, 1], I32, tag="gcol")
        nc.scalar.copy(col[:], faces3[:].rearrange("p a b -> p (a b)")[:, j : j + 1])
        nc.gpsimd.indirect_dma_start(
            out=Vg[:, j, :],
            out_offset=None,
            in_=gv_flat,
            in_offset=bass.IndirectOffsetOnAxis(ap=col[:], axis=0),
        )
    Vg4 = Vg[:].rearrange("p (ch c) d -> p ch c d", c=3)
    eA = sing.tile([128, FCH, 3], F32)
    eB = sing.tile([128, FCH, 3], F32)
    nc.vector.tensor_tensor(
        out=eA[:], in0=Vg4[:, :, 1, :], in1=Vg4[:, :, 0, :], op=A.subtract
    )
    nc.vector.tensor_tensor(
        out=eB[:], in0=Vg4[:, :, 2, :], in1=Vg4[:, :, 0, :], op=A.subtract
    )
    fn = sing.tile([128, FCH, 3], F32)
    for d in range(3):
        u, v = (d + 1) % 3, (d + 2) % 3
        t1 = work.tile([128, FCH], F32, tag="cr1")
        t2 = work.tile([128, FCH], F32, tag="cr2")
        nc.vector.tensor_tensor(out=t1[:], in0=eA[:, :, u], in1=eB[:, :, v], op=A.mult)
        nc.vector.tensor_tensor(out=t2[:], in0=eA[:, :, v], in1=eB[:, :, u], op=A.mult)
        nc.vector.tensor_tensor(out=fn[:, :, d], in0=t1[:], in1=t2[:], op=A.subtract)

    # ---------------- NN searches ----------------
    idx_gt = sing.tile([128, PC], I32)
    idx_pr = sing.tile([128, PC], I32)

    F32R = mybir.dt.float32r

    NEG = -3.0e38

    def search_chunk(rT_t, ncols, nch, idx_out, i):
        s_sb = work.tile([128, ncols], F32, tag="s")
        for c in range(nch):
            n0 = c * 512
            n1 = min(n0 + 512, ncols)
            w = n1 - n0
            ps = ppsum.tile([128, 512], F32, tag="d")
            nc.tensor.matmul(
                ps[:, 0:w],
                qT[:, i * 128 : (i + 1) * 128],
                rT_t[:, n0:n1],
                start=True,
                stop=True,
            )
            nc.scalar.copy(s_sb[:, n0:n1], ps[:, 0:w])
        # full-row max at 2x mode (fp32 SBUF single-src), in-place identity
        rmax = work.tile([128, 1], F32, tag="rmax")
        nc.vector.tensor_scalar(
            out=s_sb[:], in0=s_sb[:], scalar1=NEG, scalar2=None,
            op0=A.max, op1=A.max, accum_out=rmax[:],
        )
        mx8 = work.tile([128, 8], F32, tag="mx8")
        nc.vector.tensor_copy(mx8[:], rmax[:].to_broadcast([128, 8]))
        ix8 = work.tile([128, 8], U32, tag="ix8")
        nc.vector.max_index(ix8[:], mx8[:], s_sb[:])
        nc.vector.tensor_copy(idx_out[:, i : i + 1], ix8[:, 0:1])

    # ---------------- one-hot scatter: G[lo, c, hi] ----------------
    # emission interleaved with the NN-search chunks: the one-hot builds keep
    # the DVE busy while ScalarE evacuates search PSUM tiles.
    Gp = gpsum.tile([128, 3, 64], F32)
    _oh_state = {"k": 0}

    def emit_onehot(n):
        for _ in range(n):
            k = _oh_state["k"]
            if k >= 3 * FCH:
                return
            ch, corner = divmod(k, 3)
            ohlo = oh.tile([128, 128], FP16, tag="ohlo")
            nc.gpsimd.tensor_scalar(
                out=ohlo[:],
                in0=io128[:],
                scalar1=lo_f[:, ch : ch + 1, corner : corner + 1],
                scalar2=None,
                op0=A.is_equal,
            )
            R = oh.tile([128, 3, 64], FP16, tag="R")
            for d in range(3):
                nc.vector.tensor_scalar(
                    out=R[:, d, :],
                    in0=io64[:],
                    scalar1=hi_f[:, ch : ch + 1, corner : corner + 1],
                    scalar2=fn[:, ch : ch + 1, d : d + 1],
                    op0=A.is_equal,
                    op1=A.mult,
                )
            nc.tensor.matmul(
                Gp[:],
                ohlo[:],
                R[:],
                start=(k == 0),
                stop=(k == 3 * FCH - 1),
                skip_group_check=True,
            )
            _oh_state["k"] = k + 1

    for i in range(PC):
        search_chunk(rT, NGT, CGT, idx_gt, i)
        emit_onehot(24)
    emit_onehot(3 * FCH)  # leftovers

    # ---- unpermute column index n -> vertex id ----
    def unpermute(idx_t, mult):
        a = sing.tile([128, PC], I32, tag=f"unp_a{mult}")
        bcol = sing.tile([128, PC], I32, tag=f"unp_b{mult}")
        nc.vector.tensor_scalar(
            out=a[:], in0=idx_t[:], scalar1=127, scalar2=None, op0=A.bitwise_and
        )
        nc.vector.tensor_scalar(
            out=a[:], in0=a[:], scalar1=mult, scalar2=None, op0=A.mult
        )
        nc.vector.tensor_scalar(
            out=bcol[:], in0=idx_t[:], scalar1=7, scalar2=None, op0=A.logical_shift_right
        )
        nc.vector.tensor_tensor(out=idx_t[:], in0=a[:], in1=bcol[:], op=A.add)

    unpermute(idx_gt, 64)

    Gs = sing.tile([128, 3, 64], F32)
    nc.scalar.copy(Gs[:], Gp[:])
    Gs2 = sing.tile([128, 64, 3], F32)
    nc.vector.tensor_copy(Gs2[:], Gs[:].rearrange("p c h -> p h c"))
    nc.sync.dma_start(
        out=g_dram[:, :].rearrange("(lo hi c) one -> lo (hi c one)", lo=128, hi=64),
        in_=Gs2[:],
    )

    # gather offsets for normals: (v & 127)*192 + (v >> 7)*3
    o1 = sing.tile([128, PC], I32)
    o2 = sing.tile([128, PC], I32)
    nc.vector.tensor_scalar(
        out=o1[:], in0=idx_gt[:], scalar1=127, scalar2=None, op0=A.bitwise_and
    )
    nc.vector.tensor_scalar(
        out=o1[:], in0=o1[:], scalar1=192, scalar2=None, op0=A.mult
    )
    nc.vector.tensor_scalar(
        out=o2[:], in0=idx_gt[:], scalar1=7, scalar2=None, op0=A.logical_shift_right
    )
    nc.vector.tensor_scalar(
        out=o2[:], in0=o2[:], scalar1=3, scalar2=None, op0=A.mult
    )
    offs = sing.tile([128, PC], I32)
    nc.vector.tensor_tensor(out=offs[:], in0=o1[:], in1=o2[:], op=A.add)

    nGT = sing.tile([128, PC, 3], F32)
    for i in range(PC):
        col = gcols.tile([128, 1], I32, tag="gcol")
        nc.scalar.copy(col[:], offs[:, i : i + 1])
        nc.gpsimd.indirect_dma_start(
            out=nGT[:, i, :],
            out_offset=None,
            in_=g_dram[:, :],
            in_offset=bass.IndirectOffsetOnAxis(ap=col[:], axis=0),
        )

    for i in range(PC):
        search_chunk(rTp, VPAD, CPR, idx_pr, i)
    unpermute(idx_pr, 21)



    # ---------------- epilogue ----------------
    idx_pr3 = sing.tile([128, PC], I32)
    nc.vector.tensor_scalar(
        out=idx_pr3[:], in0=idx_pr[:], scalar1=3, scalar2=None, op0=A.mult
    )
    pv_flat2 = pv[:, :].rearrange("v (c one) -> (v c) one", one=1)
    vNN = sing.tile([128, PC, 3], F32)
    for i in range(PC):
        col = gcols.tile([128, 1], I32, tag="gcol")
        nc.scalar.copy(col[:], idx_pr3[:, i : i + 1])
        nc.gpsimd.indirect_dma_start(
            out=vNN[:, i, :],
            out_offset=None,
            in_=pv_flat2,
            in_offset=bass.IndirectOffsetOnAxis(ap=col[:], axis=0),
        )
    e = sing.tile([128, PC, 3], F32)
    nc.vector.tensor_tensor(out=e[:], in0=qRM[:], in1=vNN[:], op=A.subtract)
    tmp3 = work.tile([128, PC, 3], F32, tag="en")
    nc.vector.tensor_tensor(out=tmp3[:], in0=e[:], in1=nGT[:], op=A.mult)
    dot = sing.tile([128, PC], F32)
    nc.vector.tensor_reduce(out=dot[:], in_=tmp3[:], axis=AX.X, op=A.add)
    ee_t = work.tile([128, PC, 3], F32, tag="en")
    nc.vector.tensor_tensor(out=ee_t[:], in0=e[:], in1=e[:], op=A.mult)
    ee = sing.tile([128, PC], F32)
    nc.vector.tensor_reduce(out=ee[:], in_=ee_t[:], axis=AX.X, op=A.add)
    nn_t = work.tile([128, PC, 3], F32, tag="en")
    nc.vector.tensor_tensor(out=nn_t[:], in0=nGT[:], in1=nGT[:], op=A.mult)
    nn = sing.tile([128, PC], F32)
    nc.vector.tensor_reduce(out=nn[:], in_=nn_t[:], axis=AX.X, op=A.add)

    elen = sing.tile([128, PC], F32)
    nlen = sing.tile([128, PC], F32)
    nc.scalar.activation(elen[:], ee[:], AF.Sqrt)
    nc.scalar.activation(nlen[:], nn[:], AF.Sqrt)
    nc.vector.tensor_scalar(
        out=elen[:], in0=elen[:], scalar1=EPS, scalar2=None, op0=A.max
    )
    nc.vector.tensor_scalar(
        out=nlen[:], in0=nlen[:], scalar1=EPS, scalar2=None, op0=A.max
    )
    den = sing.tile([128, PC], F32)
    nc.vector.tensor_tensor(out=den[:], in0=elen[:], in1=nlen[:], op=A.mult)
    rden = sing.tile([128, PC], F32)
    nc.vector.reciprocal(rden[:], den[:])
    res = sing.tile([128, PC], F32)
    nc.vector.tensor_tensor(out=res[:], in0=dot[:], in1=rden[:], op=A.mult)
    partial = sing.tile([128, 1], F32)
    nc.vector.tensor_reduce(
        out=partial[:], in_=res[:], axis=AX.X, op=A.add, apply_absolute_value=True
    )
    ones = sing.tile([128, 1], F32)
    nc.vector.memset(ones[:], 1.0)
    fps = mpsum.tile([1, 1], F32, tag="fin")
    nc.tensor.matmul(fps[:], ones[:], partial[:], start=True, stop=True)
    osb = sing.tile([1, 1], F32)
    nc.scalar.copy(osb[:], fps[:])
    nc.sync.dma_start(out=out_dram[:], in_=osb[:])
    if dbg:
        nc.sync.dma_start(out=dbg["dbg_idx_gt"][:, :], in_=idx_gt[:])
        nc.sync.dma_start(out=dbg["dbg_idx_pr"][:, :], in_=idx_pr[:])
        nc.sync.dma_start(out=dbg["dbg_dot"][:, :], in_=dot[:])
        nc.sync.dma_start(out=dbg["dbg_ee"][:, :], in_=ee[:])
        nc.sync.dma_start(out=dbg["dbg_nn"][:, :], in_=nn[:])
        nc.sync.dma_start(out=dbg["dbg_res"][:, :], in_=res[:])
        nc.sync.dma_start(out=dbg["dbg_g"][:, :], in_=Gs2[:].rearrange("p a b -> p (a b)"))
        nc.sync.dma_start(out=dbg["dbg_fn"][:, :], in_=fn[:].rearrange("p a b -> p (a b)"))
        nc.sync.dma_start(out=dbg["dbg_rt"][:, :], in_=rT[:])
        nc.sync.dma_start(out=dbg["dbg_rtp"][:, :], in_=rTp[:])
        nc.sync.dma_start(out=dbg["dbg_qt"][:, :], in_=qT[:])


_NC_CACHE = None


def _get_nc():
    global _NC_CACHE
    if _NC_CACHE is None:
        _NC_CACHE = build_nc()
    return _NC_CACHE


def make_in_maps(pred_points, pred_vertices, gt_vertices, gt_faces):
    nb = pred_points.shape[0]
    faces32 = np.asarray(gt_faces).astype(np.int32, copy=False)
    return [
        dict(
            pred_points=np.ascontiguousarray(pred_points[b], dtype=np.float32),
            pred_vertices=np.ascontiguousarray(pred_vertices[b], dtype=np.float32),
            gt_vertices=np.ascontiguousarray(gt_vertices[b], dtype=np.float32),
            gt_faces32=np.ascontiguousarray(faces32[b]),
        )
        for b in range(nb)
    ]


def kernel(pred_points, pred_vertices, gt_vertices, gt_faces):
    from concourse.bass_utils import run_bass_kernel_spmd

    nb = pred_points.shape[0]
    nc = _get_nc()
    in_maps = make_in_maps(pred_points, pred_vertices, gt_vertices, gt_faces)
    res = run_bass_kernel_spmd(nc, in_maps, list(range(nb)))
    total = sum(float(res.results[i]["loss_sum"][0]) for i in range(nb))
    return np.array(total / (nb * P), dtype=np.float32)


if __name__ == "__main__":
    nc = build_nc()
    print("built ok")



# revision 16
# speedup vs baseline: 1.1640x; 1.1640x over previous
"""Trainium2 Bass kernel for nn_ChamferNormalLoss (8-core data parallel).

Sharding: pure data parallel - one batch sample per NeuronCore; the host
averages the 8 per-core |dot| sums (the only cross-core reduction).

Per-sample pipeline on each core:
  1. Brute-force NN searches (gt: 2048x8192, pred: 2048x2816-padded) as
     TensorE matmuls computing s = -d^2 = 2q.r - |r|^2 - |q|^2 with fp16
     operands split hi/lo (K=12 rows: 2q_hi.r_hi + 2q_lo.r_hi +
     2q_hi.r_lo - rsq_hi - rsq_lo - |q|^2), which makes the fp16-input
     matmul effectively fp32-accurate while streaming rows at 1 cyc/row
     (4x faster than fp32).  ScalarE evacuates each PSUM mega-tile
     [128,2048] to SBUF as fp16 (safe: -d^2 is tiny near the max).
  2. Argmax per query: a 2-level pairwise-max tree on VectorE (fp16 2x
     mode) reduces each row to per-group-of-4 maxima; one MaxIndex on the
     4x-smaller array finds the winning group; the 4 candidate vertices
     are then fetched with a batched dma_gather from a 256B-padded vertex
     array in DRAM and re-scored exactly in fp32 on VectorE.  This
     replaces a full-row MaxIndex (no fast mode: ~1 ns/elem) with tree
     passes at 0.29-0.59 ns/elem.
  3. Area-weighted vertex normals: face-corner vertices are fetched with
     two batched dma_gathers (24576 indices each, 256B elements) from the
     padded gt-vertex array; VectorE computes the cross products and
     writes fn into all 3 corner slots of the gather buffer (whose pad
     lanes are already zero); two dma_scatter_adds accumulate n[v] += fn
     into a zeroed padded array in DRAM.  SWDGE batch gathers replace the
     384 per-column indirect DMAs of the previous version (192us of Pool
     engine time -> ~20us).
  4. Epilogue: nearest gt normals / nearest pred vertices arrive via two
     more dma_gathers (query-aligned); |dot| via dot/(max(|e|,eps)*
     max(|n|,eps)); abs-sum per partition; ones-matmul partition sum.

Index plumbing: SWDGE gathers read indices "wrapped in 16 partitions"
(idx j lives at [j%16, j//16]).  All index tiles are produced with a
single partition-fold (8 small SBUF->SBUF DMAs, one per partition-octet).

Numerics: end-to-end flips vs the fp32 reference come only from fp16
score-cast collisions (~0.1% of queries); measured rel err ~1e-4.
"""

import os, sys

for _p in (
    "/opt/trn_rl_repo",
    "/opt/pypackages",
    "/root/.axon_site/_ro/trn_rl_repo",
    "/root/.axon_site/_ro/pypackages",
):
    if os.path.isdir(_p) and _p not in sys.path:
        sys.path.insert(0, _p)

import numpy as np

import concourse.bass as bass
import concourse.bacc as bacc
import concourse.tile as tile
from concourse import masks, mybir

F32 = mybir.dt.float32
FP16 = mybir.dt.float16
I32 = mybir.dt.int32
U32 = mybir.dt.uint32
I16 = mybir.dt.int16
U16 = mybir.dt.uint16
A = mybir.AluOpType
AF = mybir.ActivationFunctionType
AX = mybir.AxisListType

B = 8
P, PC = 2048, 16            # queries, outer chunks of 128
NGT = 8192                  # gt vertices
VPR, VPAD = 2562, 2816      # pred vertices, padded to 128*22
TGT, TPR = 64, 22           # vertices per partition row (gt / pred)
NF, FCH = 16384, 128        # faces, face cols per partition
BIGC = 30.0                 # pad coordinate -> -d^2 ~ -2700, never wins
EPS = 1e-12
NEG = -60000.0              # fp16-safe -inf substitute

# group-of-4 tree: group j holds columns {j + STRIDE*k}
GSTR_GT = 2048              # 8192 / 4
GSTR_PR = 704               # 2816 / 4


def build_nc():
    nc = bacc.Bacc(None, target_bir_lowering=False)
    pp = nc.dram_tensor("pred_points", [P, 3], F32, kind="ExternalInput")
    pv = nc.dram_tensor("pred_vertices", [VPR, 3], F32, kind="ExternalInput")
    gv = nc.dram_tensor("gt_vertices", [NGT, 3], F32, kind="ExternalInput")
    gf = nc.dram_tensor("gt_faces32", [NF, 3], I32, kind="ExternalInput")
    gvp = nc.dram_tensor("gv_pad", [NGT, 64], F32)
    pvp = nc.dram_tensor("pv_pad", [VPAD, 64], F32)
    ndr = nc.dram_tensor("n_pad", [NGT, 64], F32)
    out = nc.dram_tensor("loss_sum", [1], F32, kind="ExternalOutput")

    from contextlib import ExitStack

    with tile.TileContext(nc) as tc, ExitStack() as ctx:
        _body(nc, tc, ctx, pp, pv, gv, gf, gvp, pvp, ndr, out)
    nc.compile()
    return nc


def _fold16(nc, dst, src, ncols):
    """dst[q, a*8+d] = src[16d+q, a]  (wrapped-index partition fold).

    src: [128, ncols]; dst: [*, 8*ncols] (rows 0:16 written).
    """
    for d in range(8):
        nc.sync.dma_start(
            out=dst[0:16, d : 8 * ncols : 8],
            in_=src[16 * d : 16 * (d + 1), 0:ncols],
        )


def _hi_lo(nc, work, src_f32, shape, tag):
    """Return (hi16, lo16) fp16 tiles: hi = fp16(x), lo = fp16(x - hi)."""
    hi = work.tile(shape, FP16, tag=f"{tag}_hi")
    nc.vector.tensor_copy(hi[:], src_f32[:])
    hif = work.tile(shape, F32, tag=f"{tag}_hif")
    nc.vector.tensor_copy(hif[:], hi[:])
    lof = work.tile(shape, F32, tag=f"{tag}_lof")
    nc.vector.tensor_tensor(out=lof[:], in0=src_f32[:], in1=hif[:], op=A.subtract)
    lo = work.tile(shape, FP16, tag=f"{tag}_lo")
    nc.vector.tensor_copy(lo[:], lof[:])
    return hi, lo


def _body(nc, tc, ctx, pp, pv, gv, gf, gvp, pvp, ndr, out_dram):
    sing = ctx.enter_context(tc.tile_pool(name="sing", bufs=1))
    work = ctx.enter_context(tc.tile_pool(name="work", bufs=2))
    stage = ctx.enter_context(tc.tile_pool(name="stage", bufs=1))

    # ================= setup: identity, loads =================
    with tc.tile_pool(name="mpsum", bufs=2, space=bass.MemorySpace.PSUM) as mpsum:
        ident0 = sing.tile([128, 128], F32)
        masks.make_identity(nc, ident0[:])
        ident = sing.tile([128, 128], F32)
        nc.vector.tensor_copy(ident[:], ident0[:])
        ident16 = sing.tile([128, 128], FP16)
        nc.vector.tensor_copy(ident16[:], ident0[:])

        # ---- queries: qRM2[p, i, c] = pp[128*i + p]  (query q = 128i+p)
        qRM2 = sing.tile([128, PC, 3], F32)
        nc.sync.dma_start(
            out=qRM2[:], in_=pp[:, :].rearrange("(i p) c -> p i c", p=128)
        )
        qsq3 = work.tile([128, PC, 3], F32, tag="qsq3")
        nc.vector.tensor_tensor(out=qsq3[:], in0=qRM2[:], in1=qRM2[:], op=A.mult)
        qsq = sing.tile([128, PC], F32)
        nc.vector.tensor_reduce(out=qsq[:], in_=qsq3[:], axis=AX.X, op=A.add)
        q_hi, q_lo = _hi_lo(nc, work, qRM2, [128, PC, 3], "q")

        # qCM [128, 7, 16]: rows 0-2 = 2*q_hi (c-major), 3-5 = 2*q_lo, 6 = -qsq
        qCM = work.tile([128, 7, PC], FP16, tag="qcm")
        nc.vector.tensor_scalar(
            out=qCM[:, 0:3, :],
            in0=q_hi[:].rearrange("p i c -> p c i"),
            scalar1=2.0, scalar2=None, op0=A.mult,
        )
        nc.vector.tensor_scalar(
            out=qCM[:, 3:6, :],
            in0=q_lo[:].rearrange("p i c -> p c i"),
            scalar1=2.0, scalar2=None, op0=A.mult,
        )
        nc.vector.tensor_scalar(
            out=qCM[:, 6, :], in0=qsq[:], scalar1=-1.0, scalar2=None, op0=A.mult
        )
        # transpose -> qT rows: [2qh(3), 2ql(3), -|q|^2] ; cols = query 128i+p
        qT = sing.tile([12, P], FP16)
        nc.vector.memset(qT[:], -1.0)  # rows 9,10 = -1 ; others overwritten
        qtp = mpsum.tile([112, 128], FP16, tag="tp16")
        nc.tensor.transpose(qtp[:], qCM[:].rearrange("p r i -> p (r i)"), ident16[:])
        qtsb = work.tile([112, 128], FP16, tag="qtsb")
        nc.vector.tensor_copy(qtsb[:], qtp[:])
        nc.sync.dma_start(
            out=qT[0:6, :].rearrange("r (i p) -> r i p", p=128),
            in_=qtsb[0:96, :],
        )
        nc.sync.dma_start(
            out=qT[11:12, :].rearrange("r (i p) -> r i p", p=128),
            in_=qtsb[96:112, :],
        )
        nc.sync.dma_start(out=qT[6:9, :], in_=qT[0:3, :])

        # ---- gt vertices: rRM[p, t, c] = gv[64p + t]; col n = vertex
        #      (n&127)*64 + (n>>7)
        rRM = sing.tile([128, TGT, 3], F32)
        nc.sync.dma_start(out=rRM[:], in_=gv[:, :].rearrange("(p t) c -> p t c", p=128))
        rsq3 = work.tile([128, TGT, 3], F32, tag="rsq3")
        nc.vector.tensor_tensor(out=rsq3[:], in0=rRM[:], in1=rRM[:], op=A.mult)
        rsq = sing.tile([128, TGT], F32)
        nc.vector.tensor_reduce(out=rsq[:], in_=rsq3[:], axis=AX.X, op=A.add)
        r_hi, r_lo = _hi_lo(nc, work, rRM, [128, TGT, 3], "r")
        rq_hi, rq_lo = _hi_lo(nc, work, rsq, [128, TGT], "rq")

        rT = sing.tile([12, NGT], FP16)
        nc.vector.memset(rT[:], 1.0)  # row 11 = 1
        # coords hi/lo: 2 halves of 32 t-cols each -> [96,128] transposes
        for src, rows in ((r_hi, 0), (r_lo, 6)):
            for h in range(2):
                cm = work.tile([128, 3, 32], FP16, tag="rcm")
                nc.vector.tensor_copy(
                    cm[:], src[:, 32 * h : 32 * (h + 1), :].rearrange("p t c -> p c t")
                )
                tp = mpsum.tile([96, 128], FP16, tag="tp16")
                nc.tensor.transpose(tp[:], cm[:].rearrange("p c t -> p (c t)"), ident16[:])
                sb = work.tile([96, 128], FP16, tag="rtsb")
                nc.vector.tensor_copy(sb[:], tp[:])
                nc.sync.dma_start(
                    out=rT[rows : rows + 3, 4096 * h : 4096 * (h + 1)].rearrange(
                        "r (t p) -> r t p", p=128
                    ),
                    in_=sb[:],
                )
        # rows 3-5 duplicate r_hi
        nc.sync.dma_start(out=rT[3:6, :], in_=rT[0:3, :])
        # rsq hi/lo rows 9,10
        for src, row in ((rq_hi, 9), (rq_lo, 10)):
            tp = mpsum.tile([64, 128], FP16, tag="tp16")
            nc.tensor.transpose(tp[:], src[:], ident16[:])
            sb = work.tile([64, 128], FP16, tag="rtsb")
            nc.vector.tensor_copy(sb[:], tp[:])
            nc.sync.dma_start(
                out=rT[row : row + 1, :].rearrange("r (t p) -> r t p", p=128),
                in_=sb[:],
            )

        # ---- pred vertices (padded to 2816): col n = vertex (n&127)*22+(n>>7)
        rRMp = sing.tile([128, TPR, 3], F32)
        nc.vector.memset(rRMp[:], BIGC)
        rRMp_f = rRMp[:].rearrange("p t c -> p (t c)")
        pv_f = pv[:, :].rearrange("v c -> (v c)")
        nc.sync.dma_start(
            out=rRMp_f[0:116, :], in_=pv_f[0 : 116 * 66].rearrange("(p a) -> p a", a=66)
        )
        nc.sync.dma_start(
            out=rRMp_f[116:117, 0:30], in_=pv_f[116 * 66 : 116 * 66 + 30].rearrange("(o a) -> o a", o=1)
        )
        psq3 = work.tile([128, TPR, 3], F32, tag="psq3")
        nc.vector.tensor_tensor(out=psq3[:], in0=rRMp[:], in1=rRMp[:], op=A.mult)
        psq = sing.tile([128, TPR], F32)
        nc.vector.tensor_reduce(out=psq[:], in_=psq3[:], axis=AX.X, op=A.add)
        p_hi, p_lo = _hi_lo(nc, work, rRMp, [128, TPR, 3], "p")
        pq_hi, pq_lo = _hi_lo(nc, work, psq, [128, TPR], "pq")

        rTp = sing.tile([12, VPAD], FP16)
        nc.vector.memset(rTp[:], 1.0)
        for src, rows in ((p_hi, 0), (p_lo, 6)):
            cm = work.tile([128, 3, TPR], FP16, tag="pcm")
            nc.vector.tensor_copy(cm[:], src[:].rearrange("p t c -> p c t"))
            tp = mpsum.tile([66, 128], FP16, tag="tp16")
            nc.tensor.transpose(tp[:], cm[:].rearrange("p c t -> p (c t)"), ident16[:])
            sb = work.tile([66, 128], FP16, tag="ptsb")
            nc.vector.tensor_copy(sb[:], tp[:])
            nc.sync.dma_start(
                out=rTp[rows : rows + 3, :].rearrange("r (t p) -> r t p", p=128),
                in_=sb[:],
            )
        nc.sync.dma_start(out=rTp[3:6, :], in_=rTp[0:3, :])
        for src, row in ((pq_hi, 9), (pq_lo, 10)):
            tp = mpsum.tile([TPR, 128], FP16, tag="tp16")
            nc.tensor.transpose(tp[:], src[:], ident16[:])
            sb = work.tile([TPR, 128], FP16, tag="ptsb")
            nc.vector.tensor_copy(sb[:], tp[:])
            nc.sync.dma_start(
                out=rTp[row : row + 1, :].rearrange("r (t p) -> r t p", p=128),
                in_=sb[:],
            )

        # ---- padded DRAM arrays: gv_pad/pv_pad rows [x,y,z,rsq,0...]
        gstg = stage.tile([128, TGT, 64], F32, tag="gstg")
        nc.vector.memset(gstg[:], 0.0)
        nc.vector.tensor_copy(gstg[:, :, 0:3], rRM[:])
        nc.vector.tensor_copy(gstg[:, :, 3], rsq[:])
        nc.sync.dma_start(
            out=gvp[:, :].rearrange("(p t) e -> p (t e)", p=128),
            in_=gstg[:].rearrange("p t e -> p (t e)"),
        )
        pstg = stage.tile([128, TPR, 64], F32, tag="pstg")
        nc.vector.memset(pstg[:], 0.0)
        nc.vector.tensor_copy(pstg[:, :, 0:3], rRMp[:])
        nc.vector.tensor_copy(pstg[:, :, 3], psq[:])
        nc.sync.dma_start(
            out=pvp[:, :].rearrange("(p t) e -> p (t e)", p=128),
            in_=pstg[:].rearrange("p t e -> p (t e)"),
        )
        # zero n_pad
        zt = stage.tile([128, 4096], F32, tag="gstg")
        nc.vector.memset(zt[:], 0.0)
        nc.sync.dma_start(
            out=ndr[:, :].rearrange("(p t) e -> p (t e)", p=128), in_=zt[:]
        )

        # ---- faces + wrapped corner-index tiles (2 halves of 64 cols)
        faces = sing.tile([128, FCH, 3], I32)
        nc.sync.dma_start(
            out=faces[:], in_=gf[:, :].rearrange("(p ch) w -> p ch w", p=128)
        )
        Wf = []
        for h in range(2):
            F2 = sing.tile([128, 192], I16, tag=f"f2_{h}")
            nc.vector.tensor_copy(
                F2[:].rearrange("p (c t) -> p c t", c=3),
                faces[:, 64 * h : 64 * (h + 1), :].rearrange("p ch c -> p c ch"),
            )
            W = sing.tile([128, 1536], I16, tag=f"wf_{h}")
            nc.vector.memset(W[:], 0)
            _fold16(nc, W, F2, 192)
            Wf.append(W)

        # iota ramps for candidate-id math
        iot128_i = sing.tile([128, 128], I32)
        nc.gpsimd.iota(iot128_i[:], pattern=[[1, 128]], base=0, channel_multiplier=0)
        # kramp4[r, c] = c // 32  (batch idx tiles: col = 8di + d + 32k)
        kramp = sing.tile([128, 128], I32)
        nc.vector.tensor_scalar(
            out=kramp[:], in0=iot128_i[:], scalar1=5, scalar2=None,
            op0=A.logical_shift_right,
        )
        # iota8k[p, i, k] = k  for rescore select
        io4_i = sing.tile([128, PC, 4], I32)
        nc.gpsimd.iota(io4_i[:], pattern=[[0, PC], [1, 4]], base=0, channel_multiplier=0)
        io4 = sing.tile([128, PC, 4], F32)
        nc.vector.tensor_copy(io4[:], io4_i[:])

    # ================= normals: gather corners, cross, scatter ============
    # (instructions emitted up-front; tile deps let them overlap the search)
    vg_pool = ctx.enter_context(tc.tile_pool(name="vg", bufs=1))

    def normals_half(h):
        Vg = vg_pool.tile([128, 192, 64], F32, tag="vg")
        for g in range(6):
            nc.gpsimd.dma_gather(
                out_ap=Vg[:, 32 * g : 32 * (g + 1), :], in_ap=gvp[:, :],
                idxs_ap=Wf[h][:, 256 * g : 256 * (g + 1)],
                num_idxs=4096, num_idxs_reg=4096, elem_size=64,
            )
        # cross products: blocks of 64 cols per corner
        eA = work.tile([128, 64, 3], F32, tag="eA")
        eB = work.tile([128, 64, 3], F32, tag="eB")
        nc.vector.tensor_tensor(
            out=eA[:], in0=Vg[:, 64:128, 0:3], in1=Vg[:, 0:64, 0:3], op=A.subtract
        )
        nc.vector.tensor_tensor(
            out=eB[:], in0=Vg[:, 128:192, 0:3], in1=Vg[:, 0:64, 0:3], op=A.subtract
        )
        fn = work.tile([128, 64, 3], F32, tag="fn")
        for d in range(3):
            u, v = (d + 1) % 3, (d + 2) % 3
            t1 = work.tile([128, 64], F32, tag="cr1")
            t2 = work.tile([128, 64], F32, tag="cr2")
            nc.vector.tensor_tensor(out=t1[:], in0=eA[:, :, u], in1=eB[:, :, v], op=A.mult)
            nc.vector.tensor_tensor(out=t2[:], in0=eA[:, :, v], in1=eB[:, :, u], op=A.mult)
            nc.vector.tensor_tensor(out=fn[:, :, d], in0=t1[:], in1=t2[:], op=A.subtract)
        for c in range(3):
            nc.vector.tensor_copy(Vg[:, 64 * c : 64 * c + 64, 0:3], fn[:])
            # clear the rsq slot so n_pad col 3 stays clean
            nc.vector.memset(Vg[:, 64 * c : 64 * c + 64, 3], 0.0)
        for g in range(6):
            nc.gpsimd.dma_scatter_add(
                ndr[:, :], Vg[:, 32 * g : 32 * (g + 1), :],
                Wf[h][:, 256 * g : 256 * (g + 1)],
                num_idxs=4096, num_idxs_reg=4096, elem_size=64,
            )

    # ================= searches =================
    sc_pool = ctx.enter_context(tc.tile_pool(name="scores", bufs=2))

    g_gt = sing.tile([128, PC], I32)   # winning group id per (p, i)
    g_pr = sing.tile([128, PC], I32)
    v_gt = sing.tile([128, PC], I32)   # final vertex ids
    v_pr = sing.tile([128, PC], I32)

    with tc.tile_pool(name="spsum", bufs=2, space=bass.MemorySpace.PSUM) as spsum:

        def search_outer(i, rT_t, ncols, g_out):
            """One outer chunk: matmuls -> evac fp16 -> tree -> MaxIndex."""
            s16 = sc_pool.tile([128, 8192], FP16, tag="s16")
            qs = qT[:, 128 * i : 128 * (i + 1)]
            for m0 in range(0, ncols, 2048):
                mw = min(2048, ncols - m0)
                ps = spsum.tile([128, 2048], F32, tag="ps")
                for c0 in range(0, mw, 512):
                    cw = min(512, mw - c0)
                    nc.tensor.matmul(
                        ps[:, c0 : c0 + cw], qs, rT_t[:, m0 + c0 : m0 + c0 + cw],
                        start=True, stop=True,
                    )
                nc.scalar.copy(s16[:, m0 : m0 + mw], ps[:, 0:mw])
            hw = ncols // 2
            nc.vector.tensor_tensor(
                out=s16[:, 0:hw], in0=s16[:, 0:hw], in1=s16[:, hw : 2 * hw], op=A.max
            )
            qw = ncols // 4
            nc.vector.tensor_tensor(
                out=s16[:, 0:qw], in0=s16[:, 0:qw], in1=s16[:, qw : 2 * qw], op=A.max
            )
            rm = work.tile([128, 1], F32, tag="rm")
            nc.vector.tensor_scalar(
                out=s16[:, 0:qw], in0=s16[:, 0:qw], scalar1=NEG, scalar2=None,
                op0=A.max, op1=A.max, accum_out=rm[:],
            )
            mx8 = work.tile([128, 8], FP16, tag="mx8")
            nc.vector.tensor_copy(mx8[:], rm[:].to_broadcast([128, 8]))
            ix8 = work.tile([128, 8], U32, tag="ix8")
            nc.vector.max_index(ix8[:], mx8[:], s16[:, 0:qw])
            nc.vector.tensor_copy(g_out[:, i : i + 1], ix8[:, 0:1])

        with tc.tile_pool(name="vg", bufs=1) as vg_pool:

            def normals_half(h):
                Vg = vg_pool.tile([128, 192, 64], F32, tag="vg")
                for g in range(6):
                    nc.gpsimd.dma_gather(
                        out_ap=Vg[:, 32 * g : 32 * (g + 1), :], in_ap=gvp[:, :],
                        idxs_ap=Wf[h][:, 256 * g : 256 * (g + 1)],
                        num_idxs=4096, num_idxs_reg=4096, elem_size=64,
                    )
                eA = work.tile([128, 64, 3], F32, tag="eA")
                eB = work.tile([128, 64, 3], F32, tag="eB")
                nc.vector.tensor_tensor(
                    out=eA[:], in0=Vg[:, 64:128, 0:3], in1=Vg[:, 0:64, 0:3], op=A.subtract
                )
                nc.vector.tensor_tensor(
                    out=eB[:], in0=Vg[:, 128:192, 0:3], in1=Vg[:, 0:64, 0:3], op=A.subtract
                )
                fn = work.tile([128, 64, 3], F32, tag="fn")
                for d in range(3):
                    u, v = (d + 1) % 3, (d + 2) % 3
                    t1 = work.tile([128, 64], F32, tag="cr1")
                    t2 = work.tile([128, 64], F32, tag="cr2")
                    nc.vector.tensor_tensor(out=t1[:], in0=eA[:, :, u], in1=eB[:, :, v], op=A.mult)
                    nc.vector.tensor_tensor(out=t2[:], in0=eA[:, :, v], in1=eB[:, :, u], op=A.mult)
                    nc.vector.tensor_tensor(out=fn[:, :, d], in0=t1[:], in1=t2[:], op=A.subtract)
                for c in range(3):
                    nc.vector.tensor_copy(Vg[:, 64 * c : 64 * c + 64, 0:3], fn[:])
                    nc.vector.memset(Vg[:, 64 * c : 64 * c + 64, 3], 0.0)
                for g in range(6):
                    nc.gpsimd.dma_scatter_add(
                        ndr[:, :], Vg[:, 32 * g : 32 * (g + 1), :],
                        Wf[h][:, 256 * g : 256 * (g + 1)],
                        num_idxs=4096, num_idxs_reg=4096, elem_size=64,
                    )

            normals_half(0)
            for i in range(PC):
                search_outer(i, rT, NGT, g_gt)
                search_outer(i, rTp, VPAD, g_pr)
                if i == 1:
                    normals_half(1)

    def rescore_search(g_all, v_out, stride, tmul, src_d, nrows, tag):
        """Exact fp32 rescore of the 4 candidates per query for one search."""
        # candidate cols: col_k = g + stride*k ; v = (col&127)*tmul + (col>>7)
        colk = sing.tile([128, PC, 4], I32, tag=f"colk_{tag}")
        nc.vector.tensor_scalar(
            out=colk[:], in0=io4_i[:], scalar1=stride, scalar2=None, op0=A.mult
        )
        nc.vector.tensor_tensor(
            out=colk[:], in0=colk[:],
            in1=g_all[:].rearrange("p (i o) -> p i o", o=1).to_broadcast([128, PC, 4]),
            op=A.add,
        )
        vall = sing.tile([128, PC, 4], I32, tag=f"vall_{tag}")
        nc.vector.tensor_scalar(
            out=vall[:], in0=colk[:], scalar1=127, scalar2=None, op0=A.bitwise_and
        )
        nc.vector.tensor_scalar(
            out=vall[:], in0=vall[:], scalar1=tmul, scalar2=None, op0=A.mult
        )
        hi = sing.tile([128, PC, 4], I32, tag=f"hi_{tag}")
        nc.vector.tensor_scalar(
            out=hi[:], in0=colk[:], scalar1=7, scalar2=None, op0=A.logical_shift_right
        )
        nc.vector.tensor_tensor(out=vall[:], in0=vall[:], in1=hi[:], op=A.add)
        # k-major int16 copy so each fold DMA is a 1-dim run
        v16 = sing.tile([128, 4, PC], I16, tag=f"v16_{tag}")
        nc.vector.tensor_copy(v16[:], vall[:].rearrange("p i k -> p k i"))
        # wrapped idx tile: j = q + 2048k -> W[q%16, 8i + p//16 + 128k]
        Ws = sing.tile([128, 512], I16, tag=f"ws_{tag}")
        nc.vector.memset(Ws[:], 0)
        for d in range(8):
            nc.sync.dma_start(
                out=Ws[0:16, d:512:8],
                in_=v16[16 * d : 16 * (d + 1), :, :],
            )
        Vc = rsc.tile([128, 64, 64], F32, tag="vc")
        for g in range(2):
            nc.gpsimd.dma_gather(
                out_ap=Vc[:, 32 * g : 32 * (g + 1), :], in_ap=src_d[:, :],
                idxs_ap=Ws[:, 256 * g : 256 * (g + 1)],
                num_idxs=4096, num_idxs_reg=4096, elem_size=64,
            )
        # Vc[p, 16k + i, :]; score = 2 q.v - |v|^2
        prod = rsc.tile([128, 4, PC, 3], F32, tag="prod")
        nc.vector.tensor_tensor(
            out=prod[:],
            in0=Vc[:, :, 0:3].rearrange("p (k i) e -> p k i e", k=4),
            in1=qRM2[:].rearrange("p i (o e) -> p o i e", o=1).to_broadcast(
                [128, 4, PC, 3]
            ),
            op=A.mult,
        )
        dot = rsc.tile([128, 4, PC], F32, tag="dotc")
        nc.vector.tensor_reduce(out=dot[:], in_=prod[:], axis=AX.X, op=A.add)
        sc = rsc.tile([128, 4, PC], F32, tag="scc")
        nc.vector.tensor_scalar(
            out=sc[:], in0=dot[:], scalar1=2.0, scalar2=None, op0=A.mult
        )
        nc.vector.tensor_tensor(
            out=sc[:], in0=sc[:],
            in1=Vc[:, :, 3].rearrange("p (k i) -> p k i", k=4), op=A.subtract,
        )
        scd = rsc.tile([128, PC, 4], F32, tag="scd")
        nc.vector.tensor_copy(scd[:], sc[:].rearrange("p k i -> p i k"))
        mbest = rsc.tile([128, PC], F32, tag="mb")
        nc.vector.tensor_reduce(out=mbest[:], in_=scd[:], axis=AX.X, op=A.max)
        eqm = rsc.tile([128, PC, 4], F32, tag="eq")
        nc.vector.tensor_tensor(
            out=eqm[:], in0=scd[:],
            in1=mbest[:].rearrange("p (i o) -> p i o", o=1).to_broadcast([128, PC, 4]),
            op=A.is_equal,
        )
        nc.vector.tensor_tensor(out=eqm[:], in0=eqm[:], in1=io4[:], op=A.mult)
        kbest = rsc.tile([128, PC], F32, tag="kb")
        nc.vector.tensor_reduce(out=kbest[:], in_=eqm[:], axis=AX.X, op=A.add)
        kb_i = rsc.tile([128, PC], I32, tag="kbi")
        nc.vector.tensor_copy(kb_i[:], kbest[:])
        col = rsc.tile([128, PC], I32, tag="colf")
        nc.vector.tensor_scalar(
            out=col[:], in0=kb_i[:], scalar1=stride, scalar2=None, op0=A.mult
        )
        nc.vector.tensor_tensor(out=col[:], in0=col[:], in1=g_all[:], op=A.add)
        vlo = rsc.tile([128, PC], I32, tag="vlo")
        nc.vector.tensor_scalar(
            out=vlo[:], in0=col[:], scalar1=127, scalar2=None, op0=A.bitwise_and
        )
        nc.vector.tensor_scalar(
            out=vlo[:], in0=vlo[:], scalar1=tmul, scalar2=None, op0=A.mult
        )
        nc.vector.tensor_scalar(
            out=col[:], in0=col[:], scalar1=7, scalar2=None,
            op0=A.logical_shift_right,
        )
        nc.vector.tensor_tensor(out=col[:], in0=col[:], in1=vlo[:], op=A.add)
        nc.vector.tensor_scalar(
            out=col[:], in0=col[:], scalar1=0, scalar2=nrows - 1,
            op0=A.max, op1=A.min,
        )
        nc.vector.tensor_copy(v_out[:], col[:])

    rsc = ctx.enter_context(tc.tile_pool(name="rsc", bufs=1))
    rescore_search(g_gt, v_gt, GSTR_GT, TGT, gvp, NGT, "g")
    rescore_search(g_pr, v_pr, GSTR_PR, TPR, pvp, VPAD, "p")

    # ================= epilogue =================
    with tc.tile_pool(name="epsum", bufs=1, space=bass.MemorySpace.PSUM) as epsum:
        def gather_by_idx(v_all, src_d, tag):
            vi16 = sing.tile([128, PC], I16, tag=f"vi16_{tag}")
            nc.vector.tensor_copy(vi16[:], v_all[:])
            W = sing.tile([128, 128], I16, tag=f"wg_{tag}")
            nc.vector.memset(W[:], 0)
            _fold16(nc, W, vi16, PC)
            dst = sing.tile([128, PC, 64], F32, tag=f"gth_{tag}")
            nc.gpsimd.dma_gather(
                out_ap=dst[:], in_ap=src_d[:, :], idxs_ap=W[:],
                num_idxs=2048, num_idxs_reg=2048, elem_size=64,
            )
            return dst

        nrm = gather_by_idx(v_gt, ndr, "n")
        vtx = gather_by_idx(v_pr, pvp, "v")

        e = sing.tile([128, PC, 3], F32)
        nc.vector.tensor_tensor(out=e[:], in0=qRM2[:], in1=vtx[:, :, 0:3], op=A.subtract)
        tmp3 = work.tile([128, PC, 3], F32, tag="en")
        nc.vector.tensor_tensor(out=tmp3[:], in0=e[:], in1=nrm[:, :, 0:3], op=A.mult)
        dot = sing.tile([128, PC], F32)
        nc.vector.tensor_reduce(out=dot[:], in_=tmp3[:], axis=AX.X, op=A.add)
        ee_t = work.tile([128, PC, 3], F32, tag="en")
        nc.vector.tensor_tensor(out=ee_t[:], in0=e[:], in1=e[:], op=A.mult)
        ee = sing.tile([128, PC], F32)
        nc.vector.tensor_reduce(out=ee[:], in_=ee_t[:], axis=AX.X, op=A.add)
        nn_t = work.tile([128, PC, 3], F32, tag="en")
        nc.vector.tensor_tensor(
            out=nn_t[:], in0=nrm[:, :, 0:3], in1=nrm[:, :, 0:3], op=A.mult
        )
        nn = sing.tile([128, PC], F32)
        nc.vector.tensor_reduce(out=nn[:], in_=nn_t[:], axis=AX.X, op=A.add)

        elen = sing.tile([128, PC], F32)
        nlen = sing.tile([128, PC], F32)
        nc.scalar.activation(elen[:], ee[:], AF.Sqrt)
        nc.scalar.activation(nlen[:], nn[:], AF.Sqrt)
        nc.vector.tensor_scalar(out=elen[:], in0=elen[:], scalar1=EPS, scalar2=None, op0=A.max)
        nc.vector.tensor_scalar(out=nlen[:], in0=nlen[:], scalar1=EPS, scalar2=None, op0=A.max)
        den = sing.tile([128, PC], F32)
        nc.vector.tensor_tensor(out=den[:], in0=elen[:], in1=nlen[:], op=A.mult)
        rden = sing.tile([128, PC], F32)
        nc.vector.reciprocal(rden[:], den[:])
        res = sing.tile([128, PC], F32)
        nc.vector.tensor_tensor(out=res[:], in0=dot[:], in1=rden[:], op=A.mult)
        partial = sing.tile([128, 1], F32)
        nc.vector.tensor_reduce(
            out=partial[:], in_=res[:], axis=AX.X, op=A.add, apply_absolute_value=True
        )
        ones = sing.tile([128, 1], F32)
        nc.vector.memset(ones[:], 1.0)
        fps = epsum.tile([1, 1], F32, tag="fin")
        nc.tensor.matmul(fps[:], ones[:], partial[:], start=True, stop=True)
        osb = sing.tile([1, 1], F32)
        nc.scalar.copy(osb[:], fps[:])
        nc.sync.dma_start(out=out_dram[:], in_=osb[:])


_NC_CACHE = None


def _get_nc():
    global _NC_CACHE
    if _NC_CACHE is None:
        _NC_CACHE = build_nc()
    return _NC_CACHE


def make_in_maps(pred_points, pred_vertices, gt_vertices, gt_faces):
    nb = pred_points.shape[0]
    faces32 = np.asarray(gt_faces).astype(np.int32, copy=False)
    return [
        dict(
            pred_points=np.ascontiguousarray(pred_points[b], dtype=np.float32),
            pred_vertices=np.ascontiguousarray(pred_vertices[b], dtype=np.float32),
            gt_vertices=np.ascontiguousarray(gt_vertices[b], dtype=np.float32),
            gt_faces32=np.ascontiguousarray(faces32[b]),
        )
        for b in range(nb)
    ]


def kernel(pred_points, pred_vertices, gt_vertices, gt_faces):
    from concourse.bass_utils import run_bass_kernel_spmd

    nb = pred_points.shape[0]
    nc = _get_nc()
    in_maps = make_in_maps(pred_points, pred_vertices, gt_vertices, gt_faces)
    res = run_bass_kernel_spmd(nc, in_maps, list(range(nb)))
    total = sum(float(res.results[i]["loss_sum"][0]) for i in range(nb))
    return np.array(total / (nb * P), dtype=np.float32)


if __name__ == "__main__":
    nc = build_nc()
    print("built ok")


# revision 20
# speedup vs baseline: 1.5264x; 1.3113x over previous
"""Trainium2 Bass kernel for nn_ChamferNormalLoss (8-core data parallel).

Sharding: pure data parallel - one batch sample per NeuronCore; the host
averages the 8 per-core |dot| sums (the only cross-core reduction).

Per-sample pipeline on each core:
  1. Brute-force NN searches (gt: 2048x8192, pred: 2048x2816-padded) as
     TensorE matmuls computing s = -d^2 = 2q.r - |r|^2 - |q|^2 with fp16
     operands split hi/lo (K=12 rows: 2q_hi.r_hi + 2q_lo.r_hi +
     2q_hi.r_lo - rsq_hi - rsq_lo - |q|^2), which makes the fp16-input
     matmul effectively fp32-accurate while streaming rows at 1 cyc/row
     (4x faster than fp32).  ScalarE evacuates each PSUM mega-tile
     [128,2048] to SBUF as fp16 (safe: -d^2 is tiny near the max).
  2. Argmax per query: a 2-level pairwise-max tree on VectorE (fp16 2x
     mode) reduces each row to per-group-of-4 maxima; one MaxIndex on the
     4x-smaller array finds the winning group; the 4 candidate vertices
     are then fetched with a batched dma_gather from a 256B-padded vertex
     array in DRAM and re-scored exactly in fp32 on VectorE.  This
     replaces a full-row MaxIndex (no fast mode: ~1 ns/elem) with tree
     passes at 0.29-0.59 ns/elem.
  3. Area-weighted vertex normals: face-corner vertices are fetched with
     two batched dma_gathers (24576 indices each, 256B elements) from the
     padded gt-vertex array; VectorE computes the cross products and
     writes fn into all 3 corner slots of the gather buffer (whose pad
     lanes are already zero); two dma_scatter_adds accumulate n[v] += fn
     into a zeroed padded array in DRAM.  SWDGE batch gathers replace the
     384 per-column indirect DMAs of the previous version (192us of Pool
     engine time -> ~20us).
  4. Epilogue: nearest gt normals / nearest pred vertices arrive via two
     more dma_gathers (query-aligned); |dot| via dot/(max(|e|,eps)*
     max(|n|,eps)); abs-sum per partition; ones-matmul partition sum.

Index plumbing: SWDGE gathers read indices "wrapped in 16 partitions"
(idx j lives at [j%16, j//16]).  All index tiles are produced with a
single partition-fold (8 small SBUF->SBUF DMAs, one per partition-octet).

Numerics: end-to-end flips vs the fp32 reference come only from fp16
score-cast collisions (~0.1% of queries); measured rel err ~1e-4.
"""

import os, sys

for _p in (
    "/opt/trn_rl_repo",
    "/opt/pypackages",
    "/root/.axon_site/_ro/trn_rl_repo",
    "/root/.axon_site/_ro/pypackages",
):
    if os.path.isdir(_p) and _p not in sys.path:
        sys.path.insert(0, _p)

import numpy as np

import concourse.bass as bass
import concourse.bacc as bacc
import concourse.tile as tile
from concourse import masks, mybir

F32 = mybir.dt.float32
FP16 = mybir.dt.float16
I32 = mybir.dt.int32
U32 = mybir.dt.uint32
I16 = mybir.dt.int16
U16 = mybir.dt.uint16
A = mybir.AluOpType
AF = mybir.ActivationFunctionType
AX = mybir.AxisListType

B = 8
P, PC = 2048, 16            # queries, outer chunks of 128
NGT = 8192                  # gt vertices
VPR, VPAD = 2562, 2816      # pred vertices, padded to 128*22
TGT, TPR = 64, 22           # vertices per partition row (gt / pred)
NF, FCH = 16384, 128        # faces, face cols per partition
BIGC = 30.0                 # pad coordinate -> -d^2 ~ -2700, never wins
EPS = 1e-12
NEG = -60000.0              # fp16-safe -inf substitute

# group-of-4 tree: group j holds columns {j + STRIDE*k}
GSTR_GT = 2048              # 8192 / 4
GSTR_PR = 704               # 2816 / 4


def build_nc():
    nc = bacc.Bacc(None, target_bir_lowering=False)
    pp = nc.dram_tensor("pred_points", [P, 3], F32, kind="ExternalInput")
    pv = nc.dram_tensor("pred_vertices", [VPR, 3], F32, kind="ExternalInput")
    gv = nc.dram_tensor("gt_vertices", [NGT, 3], F32, kind="ExternalInput")
    gf = nc.dram_tensor("gt_faces32", [NF, 3], I32, kind="ExternalInput")
    gvp = nc.dram_tensor("gv_pad", [NGT, 64], F32)
    pvp = nc.dram_tensor("pv_pad", [VPAD, 64], F32)
    ndr = nc.dram_tensor("n_pad", [NGT, 64], F32)
    out = nc.dram_tensor("loss_sum", [1], F32, kind="ExternalOutput")

    from contextlib import ExitStack

    with tile.TileContext(nc) as tc, ExitStack() as ctx:
        _body(nc, tc, ctx, pp, pv, gv, gf, gvp, pvp, ndr, out)
    nc.compile()
    return nc


def _fold16(nc, dst, src, ncols):
    """dst[q, a*8+d] = src[16d+q, a]  (wrapped-index partition fold).

    src: [128, ncols]; dst: [*, 8*ncols] (rows 0:16 written).
    """
    for d in range(8):
        nc.sync.dma_start(
            out=dst[0:16, d : 8 * ncols : 8],
            in_=src[16 * d : 16 * (d + 1), 0:ncols],
        )


def _hi_lo(nc, work, src_f32, shape, tag):
    """Return (hi16, lo16) fp16 tiles: hi = fp16(x), lo = fp16(x - hi)."""
    hi = work.tile(shape, FP16, tag=f"{tag}_hi")
    nc.vector.tensor_copy(hi[:], src_f32[:])
    hif = work.tile(shape, F32, tag=f"{tag}_hif")
    nc.vector.tensor_copy(hif[:], hi[:])
    lof = work.tile(shape, F32, tag=f"{tag}_lof")
    nc.vector.tensor_tensor(out=lof[:], in0=src_f32[:], in1=hif[:], op=A.subtract)
    lo = work.tile(shape, FP16, tag=f"{tag}_lo")
    nc.vector.tensor_copy(lo[:], lof[:])
    return hi, lo


def _body(nc, tc, ctx, pp, pv, gv, gf, gvp, pvp, ndr, out_dram):
    sing = ctx.enter_context(tc.tile_pool(name="sing", bufs=1))
    work = ctx.enter_context(tc.tile_pool(name="work", bufs=2))
    stage = ctx.enter_context(tc.tile_pool(name="stage", bufs=1))

    # ================= setup: identity, loads =================
    with tc.tile_pool(name="mpsum", bufs=2, space=bass.MemorySpace.PSUM) as mpsum:
        ident0 = sing.tile([128, 128], F32)
        masks.make_identity(nc, ident0[:])
        ident = sing.tile([128, 128], F32)
        nc.vector.tensor_copy(ident[:], ident0[:])
        ident16 = sing.tile([128, 128], FP16)
        nc.vector.tensor_copy(ident16[:], ident0[:])

        # ---- queries: qRM2[p, i, c] = pp[128*i + p]  (query q = 128i+p)
        qRM2 = sing.tile([128, PC, 3], F32)
        nc.sync.dma_start(
            out=qRM2[:], in_=pp[:, :].rearrange("(i p) c -> p i c", p=128)
        )
        qsq3 = work.tile([128, PC, 3], F32, tag="qsq3")
        nc.vector.tensor_tensor(out=qsq3[:], in0=qRM2[:], in1=qRM2[:], op=A.mult)
        qsq = sing.tile([128, PC], F32)
        nc.vector.tensor_reduce(out=qsq[:], in_=qsq3[:], axis=AX.X, op=A.add)
        q_hi, q_lo = _hi_lo(nc, work, qRM2, [128, PC, 3], "q")

        # qCM [128, 7, 16]: rows 0-2 = 2*q_hi (c-major), 3-5 = 2*q_lo, 6 = -qsq
        qCM = work.tile([128, 7, PC], FP16, tag="qcm")
        nc.vector.tensor_scalar(
            out=qCM[:, 0:3, :],
            in0=q_hi[:].rearrange("p i c -> p c i"),
            scalar1=2.0, scalar2=None, op0=A.mult,
        )
        nc.vector.tensor_scalar(
            out=qCM[:, 3:6, :],
            in0=q_lo[:].rearrange("p i c -> p c i"),
            scalar1=2.0, scalar2=None, op0=A.mult,
        )
        nc.vector.tensor_scalar(
            out=qCM[:, 6, :], in0=qsq[:], scalar1=-1.0, scalar2=None, op0=A.mult
        )
        # transpose -> qT rows: [2qh(3), 2ql(3), -|q|^2] ; cols = query 128i+p
        qT = sing.tile([12, P], FP16)
        nc.vector.memset(qT[:], -1.0)  # rows 9,10 = -1 ; others overwritten
        qtp = mpsum.tile([112, 128], FP16, tag="tp16")
        nc.tensor.transpose(qtp[:], qCM[:].rearrange("p r i -> p (r i)"), ident16[:])
        qtsb = work.tile([112, 128], FP16, tag="qtsb")
        nc.vector.tensor_copy(qtsb[:], qtp[:])
        nc.sync.dma_start(
            out=qT[0:6, :].rearrange("r (i p) -> r i p", p=128),
            in_=qtsb[0:96, :],
        )
        nc.sync.dma_start(
            out=qT[11:12, :].rearrange("r (i p) -> r i p", p=128),
            in_=qtsb[96:112, :],
        )
        nc.sync.dma_start(out=qT[6:9, :], in_=qT[0:3, :])

        # ---- gt vertices: rRM[p, t, c] = gv[64p + t]; col n = vertex
        #      (n&127)*64 + (n>>7)
        rRM = sing.tile([128, TGT, 3], F32)
        nc.sync.dma_start(out=rRM[:], in_=gv[:, :].rearrange("(p t) c -> p t c", p=128))
        rsq3 = work.tile([128, TGT, 3], F32, tag="rsq3")
        nc.vector.tensor_tensor(out=rsq3[:], in0=rRM[:], in1=rRM[:], op=A.mult)
        rsq = sing.tile([128, TGT], F32)
        nc.vector.tensor_reduce(out=rsq[:], in_=rsq3[:], axis=AX.X, op=A.add)
        r_hi, r_lo = _hi_lo(nc, work, rRM, [128, TGT, 3], "r")
        rq_hi, rq_lo = _hi_lo(nc, work, rsq, [128, TGT], "rq")

        rT = sing.tile([12, NGT], FP16)
        nc.vector.memset(rT[:], 1.0)  # row 11 = 1
        # coords hi/lo: 2 halves of 32 t-cols each -> [96,128] transposes
        for src, rows in ((r_hi, 0), (r_lo, 6)):
            for h in range(2):
                cm = work.tile([128, 3, 32], FP16, tag="rcm")
                nc.vector.tensor_copy(
                    cm[:], src[:, 32 * h : 32 * (h + 1), :].rearrange("p t c -> p c t")
                )
                tp = mpsum.tile([96, 128], FP16, tag="tp16")
                nc.tensor.transpose(tp[:], cm[:].rearrange("p c t -> p (c t)"), ident16[:])
                sb = work.tile([96, 128], FP16, tag="rtsb")
                nc.vector.tensor_copy(sb[:], tp[:])
                nc.sync.dma_start(
                    out=rT[rows : rows + 3, 4096 * h : 4096 * (h + 1)].rearrange(
                        "r (t p) -> r t p", p=128
                    ),
                    in_=sb[:],
                )
        # rows 3-5 duplicate r_hi
        nc.sync.dma_start(out=rT[3:6, :], in_=rT[0:3, :])
        # rsq hi/lo rows 9,10
        for src, row in ((rq_hi, 9), (rq_lo, 10)):
            tp = mpsum.tile([64, 128], FP16, tag="tp16")
            nc.tensor.transpose(tp[:], src[:], ident16[:])
            sb = work.tile([64, 128], FP16, tag="rtsb")
            nc.vector.tensor_copy(sb[:], tp[:])
            nc.sync.dma_start(
                out=rT[row : row + 1, :].rearrange("r (t p) -> r t p", p=128),
                in_=sb[:],
            )

        # ---- pred vertices (padded to 2816): col n = vertex (n&127)*22+(n>>7)
        rRMp = sing.tile([128, TPR, 3], F32)
        nc.vector.memset(rRMp[:], BIGC)
        rRMp_f = rRMp[:].rearrange("p t c -> p (t c)")
        pv_f = pv[:, :].rearrange("v c -> (v c)")
        nc.sync.dma_start(
            out=rRMp_f[0:116, :], in_=pv_f[0 : 116 * 66].rearrange("(p a) -> p a", a=66)
        )
        nc.sync.dma_start(
            out=rRMp_f[116:117, 0:30], in_=pv_f[116 * 66 : 116 * 66 + 30].rearrange("(o a) -> o a", o=1)
        )
        psq3 = work.tile([128, TPR, 3], F32, tag="psq3")
        nc.vector.tensor_tensor(out=psq3[:], in0=rRMp[:], in1=rRMp[:], op=A.mult)
        psq = sing.tile([128, TPR], F32)
        nc.vector.tensor_reduce(out=psq[:], in_=psq3[:], axis=AX.X, op=A.add)
        p_hi, p_lo = _hi_lo(nc, work, rRMp, [128, TPR, 3], "p")
        pq_hi, pq_lo = _hi_lo(nc, work, psq, [128, TPR], "pq")

        rTp = sing.tile([12, VPAD], FP16)
        nc.vector.memset(rTp[:], 1.0)
        for src, rows in ((p_hi, 0), (p_lo, 6)):
            cm = work.tile([128, 3, TPR], FP16, tag="pcm")
            nc.vector.tensor_copy(cm[:], src[:].rearrange("p t c -> p c t"))
            tp = mpsum.tile([66, 128], FP16, tag="tp16")
            nc.tensor.transpose(tp[:], cm[:].rearrange("p c t -> p (c t)"), ident16[:])
            sb = work.tile([66, 128], FP16, tag="ptsb")
            nc.vector.tensor_copy(sb[:], tp[:])
            nc.sync.dma_start(
                out=rTp[rows : rows + 3, :].rearrange("r (t p) -> r t p", p=128),
                in_=sb[:],
            )
        nc.sync.dma_start(out=rTp[3:6, :], in_=rTp[0:3, :])
        for src, row in ((pq_hi, 9), (pq_lo, 10)):
            tp = mpsum.tile([TPR, 128], FP16, tag="tp16")
            nc.tensor.transpose(tp[:], src[:], ident16[:])
            sb = work.tile([TPR, 128], FP16, tag="ptsb")
            nc.vector.tensor_copy(sb[:], tp[:])
            nc.sync.dma_start(
                out=rTp[row : row + 1, :].rearrange("r (t p) -> r t p", p=128),
                in_=sb[:],
            )

        # ---- padded DRAM arrays: gv_pad/pv_pad rows [x,y,z,rsq,0...]
        gstg = stage.tile([128, TGT, 64], F32, tag="gstg")
        nc.gpsimd.memset(gstg[:], 0.0)
        nc.vector.tensor_copy(gstg[:, :, 0:3], rRM[:])
        nc.vector.tensor_copy(gstg[:, :, 3], rsq[:])
        nc.sync.dma_start(
            out=gvp[:, :].rearrange("(p t) e -> p (t e)", p=128),
            in_=gstg[:].rearrange("p t e -> p (t e)"),
        )
        pstg = stage.tile([128, TPR, 64], F32, tag="pstg")
        nc.gpsimd.memset(pstg[:], 0.0)
        nc.vector.tensor_copy(pstg[:, :, 0:3], rRMp[:])
        nc.vector.tensor_copy(pstg[:, :, 3], psq[:])
        nc.sync.dma_start(
            out=pvp[:, :].rearrange("(p t) e -> p (t e)", p=128),
            in_=pstg[:].rearrange("p t e -> p (t e)"),
        )
        # zero n_pad
        zt = stage.tile([128, 4096], F32, tag="gstg")
        nc.gpsimd.memset(zt[:], 0.0)
        nc.sync.dma_start(
            out=ndr[:, :].rearrange("(p t) e -> p (t e)", p=128), in_=zt[:]
        )

        # ---- faces + wrapped corner-index tiles (2 halves of 64 cols)
        faces = sing.tile([128, FCH, 3], I32)
        nc.sync.dma_start(
            out=faces[:], in_=gf[:, :].rearrange("(p ch) w -> p ch w", p=128)
        )
        Wf = []
        for h in range(2):
            F2 = sing.tile([128, 192], I16, tag=f"f2_{h}")
            nc.vector.tensor_copy(
                F2[:].rearrange("p (c t) -> p c t", c=3),
                faces[:, 64 * h : 64 * (h + 1), :].rearrange("p ch c -> p c ch"),
            )
            W = sing.tile([128, 1536], I16, tag=f"wf_{h}")
            nc.vector.memset(W[:], 0)
            _fold16(nc, W, F2, 192)
            Wf.append(W)

        # iota ramps for candidate-id math
        iot128_i = sing.tile([128, 128], I32)
        nc.gpsimd.iota(iot128_i[:], pattern=[[1, 128]], base=0, channel_multiplier=0)
        # kramp4[r, c] = c // 32  (batch idx tiles: col = 8di + d + 32k)
        kramp = sing.tile([128, 128], I32)
        nc.vector.tensor_scalar(
            out=kramp[:], in0=iot128_i[:], scalar1=5, scalar2=None,
            op0=A.logical_shift_right,
        )
        # iota8k[p, i, k] = k  for rescore select
        io4_i = sing.tile([128, PC, 4], I32)
        nc.gpsimd.iota(io4_i[:], pattern=[[0, PC], [1, 4]], base=0, channel_multiplier=0)
        io4 = sing.tile([128, PC, 4], F32)
        nc.vector.tensor_copy(io4[:], io4_i[:])

    # ================= normals: gather corners, cross, scatter ============
    # (instructions emitted up-front; tile deps let them overlap the search)
    vg_pool = ctx.enter_context(tc.tile_pool(name="vg", bufs=1))

    def normals_half(h):
        Vg = vg_pool.tile([128, 192, 64], F32, tag="vg")
        for g in range(6):
            nc.gpsimd.dma_gather(
                out_ap=Vg[:, 32 * g : 32 * (g + 1), :], in_ap=gvp[:, :],
                idxs_ap=Wf[h][:, 256 * g : 256 * (g + 1)],
                num_idxs=4096, num_idxs_reg=4096, elem_size=64,
            )
        # cross products: blocks of 64 cols per corner
        eA = work.tile([128, 64, 3], F32, tag="eA")
        eB = work.tile([128, 64, 3], F32, tag="eB")
        nc.vector.tensor_tensor(
            out=eA[:], in0=Vg[:, 64:128, 0:3], in1=Vg[:, 0:64, 0:3], op=A.subtract
        )
        nc.vector.tensor_tensor(
            out=eB[:], in0=Vg[:, 128:192, 0:3], in1=Vg[:, 0:64, 0:3], op=A.subtract
        )
        fn = work.tile([128, 64, 3], F32, tag="fn")
        for d in range(3):
            u, v = (d + 1) % 3, (d + 2) % 3
            t1 = work.tile([128, 64], F32, tag="cr1")
            t2 = work.tile([128, 64], F32, tag="cr2")
            nc.vector.tensor_tensor(out=t1[:], in0=eA[:, :, u], in1=eB[:, :, v], op=A.mult)
            nc.vector.tensor_tensor(out=t2[:], in0=eA[:, :, v], in1=eB[:, :, u], op=A.mult)
            nc.vector.tensor_tensor(out=fn[:, :, d], in0=t1[:], in1=t2[:], op=A.subtract)
        for c in range(3):
            nc.vector.tensor_copy(Vg[:, 64 * c : 64 * c + 64, 0:3], fn[:])
            # clear the rsq slot so n_pad col 3 stays clean
            nc.vector.memset(Vg[:, 64 * c : 64 * c + 64, 3], 0.0)
        for g in range(6):
            nc.gpsimd.dma_scatter_add(
                ndr[:, :], Vg[:, 32 * g : 32 * (g + 1), :],
                Wf[h][:, 256 * g : 256 * (g + 1)],
                num_idxs=4096, num_idxs_reg=4096, elem_size=64,
            )

    # ================= searches =================
    sc_pool = ctx.enter_context(tc.tile_pool(name="scores", bufs=3))

    g_gt = sing.tile([128, PC], I32)   # winning group id per (p, i)
    g_pr = sing.tile([128, PC], I32)
    v_gt = sing.tile([128, PC], I32)   # final vertex ids
    v_pr = sing.tile([128, PC], I32)

    with tc.tile_pool(name="spsum", bufs=2, space=bass.MemorySpace.PSUM) as spsum:

        def search_outer(i, rT_t, ncols, g_out):
            """One outer chunk: matmuls -> evac fp16 -> tree -> MaxIndex."""
            s16 = sc_pool.tile([128, 8192], FP16, tag="s16")
            qs = qT[:, 128 * i : 128 * (i + 1)]
            for m0 in range(0, ncols, 2048):
                mw = min(2048, ncols - m0)
                ps = spsum.tile([128, 2048], F32, tag="ps")
                for c0 in range(0, mw, 512):
                    cw = min(512, mw - c0)
                    nc.tensor.matmul(
                        ps[:, c0 : c0 + cw], qs, rT_t[:, m0 + c0 : m0 + c0 + cw],
                        start=True, stop=True,
                    )
                nc.scalar.copy(s16[:, m0 : m0 + mw], ps[:, 0:mw])
            qw = ncols // 4
            # pair adjacent quarters first so level-1a only needs megas 0-1
            nc.vector.tensor_tensor(
                out=s16[:, 0:qw], in0=s16[:, 0:qw], in1=s16[:, qw : 2 * qw], op=A.max
            )
            nc.vector.tensor_tensor(
                out=s16[:, 2 * qw : 3 * qw], in0=s16[:, 2 * qw : 3 * qw],
                in1=s16[:, 3 * qw : 4 * qw], op=A.max,
            )
            nc.vector.tensor_tensor(
                out=s16[:, 0:qw], in0=s16[:, 0:qw], in1=s16[:, 2 * qw : 3 * qw],
                op=A.max,
            )
            rm = work.tile([128, 1], F32, tag="rm")
            nc.vector.tensor_scalar(
                out=s16[:, 0:qw], in0=s16[:, 0:qw], scalar1=NEG, scalar2=None,
                op0=A.max, op1=A.max, accum_out=rm[:],
            )
            mx8 = work.tile([128, 8], FP16, tag="mx8")
            nc.vector.tensor_copy(mx8[:], rm[:].to_broadcast([128, 8]))
            ix8 = work.tile([128, 8], U32, tag="ix8")
            nc.vector.max_index(ix8[:], mx8[:], s16[:, 0:qw])
            nc.vector.tensor_copy(g_out[:, i : i + 1], ix8[:, 0:1])

        vg_pool = tc.alloc_tile_pool(name="vg", bufs=1)

        def normals_half(h):
            Vg = vg_pool.tile([128, 192, 64], F32, tag="vg")
            for g in range(6):
                nc.gpsimd.dma_gather(
                    out_ap=Vg[:, 32 * g : 32 * (g + 1), :], in_ap=gvp[:, :],
                    idxs_ap=Wf[h][:, 256 * g : 256 * (g + 1)],
                    num_idxs=4096, num_idxs_reg=4096, elem_size=64,
                )
            eA = work.tile([128, 64, 3], F32, tag="eA")
            eB = work.tile([128, 64, 3], F32, tag="eB")
            nc.vector.tensor_tensor(
                out=eA[:], in0=Vg[:, 64:128, 0:3], in1=Vg[:, 0:64, 0:3], op=A.subtract
            )
            nc.vector.tensor_tensor(
                out=eB[:], in0=Vg[:, 128:192, 0:3], in1=Vg[:, 0:64, 0:3], op=A.subtract
            )
            fn = work.tile([128, 64, 3], F32, tag="fn")
            for d in range(3):
                u, v = (d + 1) % 3, (d + 2) % 3
                t1 = work.tile([128, 64], F32, tag="cr1")
                t2 = work.tile([128, 64], F32, tag="cr2")
                nc.vector.tensor_tensor(out=t1[:], in0=eA[:, :, u], in1=eB[:, :, v], op=A.mult)
                nc.vector.tensor_tensor(out=t2[:], in0=eA[:, :, v], in1=eB[:, :, u], op=A.mult)
                nc.vector.tensor_tensor(out=fn[:, :, d], in0=t1[:], in1=t2[:], op=A.subtract)
            for c in range(3):
                nc.vector.tensor_copy(Vg[:, 64 * c : 64 * c + 64, 0:3], fn[:])
                nc.vector.memset(Vg[:, 64 * c : 64 * c + 64, 3], 0.0)
            for g in range(6):
                nc.gpsimd.dma_scatter_add(
                    ndr[:, :], Vg[:, 32 * g : 32 * (g + 1), :],
                    Wf[h][:, 256 * g : 256 * (g + 1)],
                    num_idxs=4096, num_idxs_reg=4096, elem_size=64,
                )

        def rescore_search(rsc, g_all, v_out, stride, tmul, src_d, nrows, tag):
            """Exact fp32 rescore of the 4 candidates per query for one search."""
            colk = sing.tile([128, PC, 4], I32, tag=f"colk_{tag}")
            nc.vector.tensor_scalar(
                out=colk[:], in0=io4_i[:], scalar1=stride, scalar2=None, op0=A.mult
            )
            nc.vector.tensor_tensor(
                out=colk[:], in0=colk[:],
                in1=g_all[:].rearrange("p (i o) -> p i o", o=1).to_broadcast([128, PC, 4]),
                op=A.add,
            )
            vall = sing.tile([128, PC, 4], I32, tag=f"vall_{tag}")
            nc.vector.tensor_scalar(
                out=vall[:], in0=colk[:], scalar1=127, scalar2=None, op0=A.bitwise_and
            )
            nc.vector.tensor_scalar(
                out=vall[:], in0=vall[:], scalar1=tmul, scalar2=None, op0=A.mult
            )
            hi = sing.tile([128, PC, 4], I32, tag=f"hi_{tag}")
            nc.vector.tensor_scalar(
                out=hi[:], in0=colk[:], scalar1=7, scalar2=None, op0=A.logical_shift_right
            )
            nc.vector.tensor_tensor(out=vall[:], in0=vall[:], in1=hi[:], op=A.add)
            # k-major int16 copy so each fold DMA is a 1-dim run
            v16 = sing.tile([128, 4, PC], I16, tag=f"v16_{tag}")
            nc.vector.tensor_copy(v16[:], vall[:].rearrange("p i k -> p k i"))
            Ws = sing.tile([128, 512], I16, tag=f"ws_{tag}")
            nc.vector.memset(Ws[:], 0)
            for d in range(8):
                nc.sync.dma_start(
                    out=Ws[0:16, d:512:8],
                    in_=v16[16 * d : 16 * (d + 1), :, :],
                )
            Vc = rsc.tile([128, 64, 64], F32, tag="vc")
            for g in range(2):
                nc.gpsimd.dma_gather(
                    out_ap=Vc[:, 32 * g : 32 * (g + 1), :], in_ap=src_d[:, :],
                    idxs_ap=Ws[:, 256 * g : 256 * (g + 1)],
                    num_idxs=4096, num_idxs_reg=4096, elem_size=64,
                )
            # Vc[p, 16k + i, :]; score = 2 q.v - |v|^2
            prod = rsc.tile([128, 4, PC, 3], F32, tag="prod")
            nc.vector.tensor_tensor(
                out=prod[:],
                in0=Vc[:, :, 0:3].rearrange("p (k i) e -> p k i e", k=4),
                in1=qRM2[:].rearrange("p i (o e) -> p o i e", o=1).to_broadcast(
                    [128, 4, PC, 3]
                ),
                op=A.mult,
            )
            dot = rsc.tile([128, 4, PC], F32, tag="dotc")
            nc.vector.tensor_reduce(out=dot[:], in_=prod[:], axis=AX.X, op=A.add)
            sc = rsc.tile([128, 4, PC], F32, tag="scc")
            nc.vector.tensor_scalar(
                out=sc[:], in0=dot[:], scalar1=2.0, scalar2=None, op0=A.mult
            )
            nc.vector.tensor_tensor(
                out=sc[:], in0=sc[:],
                in1=Vc[:, :, 3].rearrange("p (k i) -> p k i", k=4), op=A.subtract,
            )
            scd = rsc.tile([128, PC, 4], F32, tag="scd")
            nc.vector.tensor_copy(scd[:], sc[:].rearrange("p k i -> p i k"))
            mbest = rsc.tile([128, PC], F32, tag="mb")
            nc.vector.tensor_reduce(out=mbest[:], in_=scd[:], axis=AX.X, op=A.max)
            eqm = rsc.tile([128, PC, 4], F32, tag="eq")
            nc.vector.tensor_tensor(
                out=eqm[:], in0=scd[:],
                in1=mbest[:].rearrange("p (i o) -> p i o", o=1).to_broadcast([128, PC, 4]),
                op=A.is_equal,
            )
            nc.vector.tensor_tensor(out=eqm[:], in0=eqm[:], in1=io4[:], op=A.mult)
            kbest = rsc.tile([128, PC], F32, tag="kb")
            nc.vector.tensor_reduce(out=kbest[:], in_=eqm[:], axis=AX.X, op=A.add)
            kb_i = rsc.tile([128, PC], I32, tag="kbi")
            nc.vector.tensor_copy(kb_i[:], kbest[:])
            col = rsc.tile([128, PC], I32, tag="colf")
            nc.vector.tensor_scalar(
                out=col[:], in0=kb_i[:], scalar1=stride, scalar2=None, op0=A.mult
            )
            nc.vector.tensor_tensor(out=col[:], in0=col[:], in1=g_all[:], op=A.add)
            vlo = rsc.tile([128, PC], I32, tag="vlo")
            nc.vector.tensor_scalar(
                out=vlo[:], in0=col[:], scalar1=127, scalar2=None, op0=A.bitwise_and
            )
            nc.vector.tensor_scalar(
                out=vlo[:], in0=vlo[:], scalar1=tmul, scalar2=None, op0=A.mult
            )
            nc.vector.tensor_scalar(
                out=col[:], in0=col[:], scalar1=7, scalar2=None,
                op0=A.logical_shift_right,
            )
            nc.vector.tensor_tensor(out=col[:], in0=col[:], in1=vlo[:], op=A.add)
            nc.vector.tensor_scalar(
                out=col[:], in0=col[:], scalar1=0, scalar2=nrows - 1,
                op0=A.max, op1=A.min,
            )
            nc.vector.tensor_copy(v_out[:], col[:])

        normals_half(0)
        for i in range(PC):
            search_outer(i, rT, NGT, g_gt)
            if i == 1:
                normals_half(1)
            if i == 2:
                vg_pool.release()   # free 48KB for the rescore pool

        rsc = ctx.enter_context(tc.tile_pool(name="rsc", bufs=1))
        rescore_search(rsc, g_gt, v_gt, GSTR_GT, TGT, gvp, NGT, "g")
        for i in range(PC):
            search_outer(i, rTp, VPAD, g_pr)
        rescore_search(rsc, g_pr, v_pr, GSTR_PR, TPR, pvp, VPAD, "p")


    # ================= epilogue =================
    with tc.tile_pool(name="epsum", bufs=1, space=bass.MemorySpace.PSUM) as epsum:
        def gather_by_idx(v_all, src_d, tag):
            vi16 = sing.tile([128, PC], I16, tag=f"vi16_{tag}")
            nc.vector.tensor_copy(vi16[:], v_all[:])
            W = sing.tile([128, 128], I16, tag=f"wg_{tag}")
            nc.vector.memset(W[:], 0)
            _fold16(nc, W, vi16, PC)
            dst = sing.tile([128, PC, 64], F32, tag=f"gth_{tag}")
            nc.gpsimd.dma_gather(
                out_ap=dst[:], in_ap=src_d[:, :], idxs_ap=W[:],
                num_idxs=2048, num_idxs_reg=2048, elem_size=64,
            )
            return dst

        nrm = gather_by_idx(v_gt, ndr, "n")
        vtx = gather_by_idx(v_pr, pvp, "v")

        e = sing.tile([128, PC, 3], F32)
        nc.vector.tensor_tensor(out=e[:], in0=qRM2[:], in1=vtx[:, :, 0:3], op=A.subtract)
        tmp3 = work.tile([128, PC, 3], F32, tag="en")
        nc.vector.tensor_tensor(out=tmp3[:], in0=e[:], in1=nrm[:, :, 0:3], op=A.mult)
        dot = sing.tile([128, PC], F32)
        nc.vector.tensor_reduce(out=dot[:], in_=tmp3[:], axis=AX.X, op=A.add)
        ee_t = work.tile([128, PC, 3], F32, tag="en")
        nc.vector.tensor_tensor(out=ee_t[:], in0=e[:], in1=e[:], op=A.mult)
        ee = sing.tile([128, PC], F32)
        nc.vector.tensor_reduce(out=ee[:], in_=ee_t[:], axis=AX.X, op=A.add)
        nn_t = work.tile([128, PC, 3], F32, tag="en")
        nc.vector.tensor_tensor(
            out=nn_t[:], in0=nrm[:, :, 0:3], in1=nrm[:, :, 0:3], op=A.mult
        )
        nn = sing.tile([128, PC], F32)
        nc.vector.tensor_reduce(out=nn[:], in_=nn_t[:], axis=AX.X, op=A.add)

        elen = sing.tile([128, PC], F32)
        nlen = sing.tile([128, PC], F32)
        nc.scalar.activation(elen[:], ee[:], AF.Sqrt)
        nc.scalar.activation(nlen[:], nn[:], AF.Sqrt)
        nc.vector.tensor_scalar(out=elen[:], in0=elen[:], scalar1=EPS, scalar2=None, op0=A.max)
        nc.vector.tensor_scalar(out=nlen[:], in0=nlen[:], scalar1=EPS, scalar2=None, op0=A.max)
        den = sing.tile([128, PC], F32)
        nc.vector.tensor_tensor(out=den[:], in0=elen[:], in1=nlen[:], op=A.mult)
        rden = sing.tile([128, PC], F32)
        nc.vector.reciprocal(rden[:], den[:])
        res = sing.tile([128, PC], F32)
        nc.vector.tensor_tensor(out=res[:], in0=dot[:], in1=rden[:], op=A.mult)
        partial = sing.tile([128, 1], F32)
        nc.vector.tensor_reduce(
            out=partial[:], in_=res[:], axis=AX.X, op=A.add, apply_absolute_value=True
        )
        ones = sing.tile([128, 1], F32)
        nc.vector.memset(ones[:], 1.0)
        fps = epsum.tile([1, 1], F32, tag="fin")
        nc.tensor.matmul(fps[:], ones[:], partial[:], start=True, stop=True)
        osb = sing.tile([1, 1], F32)
        nc.scalar.copy(osb[:], fps[:])
        nc.sync.dma_start(out=out_dram[:], in_=osb[:])


_NC_CACHE = None


def _get_nc():
    global _NC_CACHE
    if _NC_CACHE is None:
        _NC_CACHE = build_nc()
    return _NC_CACHE


def make_in_maps(pred_points, pred_vertices, gt_vertices, gt_faces):
    nb = pred_points.shape[0]
    faces32 = np.asarray(gt_faces).astype(np.int32, copy=False)
    return [
        dict(
            pred_points=np.ascontiguousarray(pred_points[b], dtype=np.float32),
            pred_vertices=np.ascontiguousarray(pred_vertices[b], dtype=np.float32),
            gt_vertices=np.ascontiguousarray(gt_vertices[b], dtype=np.float32),
            gt_faces32=np.ascontiguousarray(faces32[b]),
        )
        for b in range(nb)
    ]


def kernel(pred_points, pred_vertices, gt_vertices, gt_faces):
    from concourse.bass_utils import run_bass_kernel_spmd

    nb = pred_points.shape[0]
    nc = _get_nc()
    in_maps = make_in_maps(pred_points, pred_vertices, gt_vertices, gt_faces)
    res = run_bass_kernel_spmd(nc, in_maps, list(range(nb)))
    total = sum(float(res.results[i]["loss_sum"][0]) for i in range(nb))
    return np.array(total / (nb * P), dtype=np.float32)


if __name__ == "__main__":
    nc = build_nc()
    print("built ok")


# revision 22
# speedup vs baseline: 1.5516x; 1.0165x over previous
"""Trainium2 Bass kernel for nn_ChamferNormalLoss (8-core data parallel).

Sharding: pure data parallel - one batch sample per NeuronCore; the host
averages the 8 per-core |dot| sums (the only cross-core reduction).

Per-sample pipeline on each core:
  1. Brute-force NN searches (gt: 2048x8192, pred: 2048x2816-padded) as
     TensorE matmuls computing s = -d^2 = 2q.r - |r|^2 - |q|^2 with fp16
     operands split hi/lo (K=12 rows: 2q_hi.r_hi + 2q_lo.r_hi +
     2q_hi.r_lo - rsq_hi - rsq_lo - |q|^2), which makes the fp16-input
     matmul effectively fp32-accurate while streaming rows at 1 cyc/row
     (4x faster than fp32).  ScalarE evacuates each PSUM mega-tile
     [128,2048] to SBUF as fp16 (safe: -d^2 is tiny near the max).
  2. Argmax per query: a 2-level pairwise-max tree on VectorE (fp16 2x
     mode) reduces each row to per-group-of-4 maxima; one MaxIndex on the
     4x-smaller array finds the winning group; the 4 candidate vertices
     are then fetched with a batched dma_gather from a 256B-padded vertex
     array in DRAM and re-scored exactly in fp32 on VectorE.  This
     replaces a full-row MaxIndex (no fast mode: ~1 ns/elem) with tree
     passes at 0.29-0.59 ns/elem.
  3. Area-weighted vertex normals: face-corner vertices are fetched with
     two batched dma_gathers (24576 indices each, 256B elements) from the
     padded gt-vertex array; VectorE computes the cross products and
     writes fn into all 3 corner slots of the gather buffer (whose pad
     lanes are already zero); two dma_scatter_adds accumulate n[v] += fn
     into a zeroed padded array in DRAM.  SWDGE batch gathers replace the
     384 per-column indirect DMAs of the previous version (192us of Pool
     engine time -> ~20us).
  4. Epilogue: nearest gt normals / nearest pred vertices arrive via two
     more dma_gathers (query-aligned); |dot| via dot/(max(|e|,eps)*
     max(|n|,eps)); abs-sum per partition; ones-matmul partition sum.

Index plumbing: SWDGE gathers read indices "wrapped in 16 partitions"
(idx j lives at [j%16, j//16]).  All index tiles are produced with a
single partition-fold (8 small SBUF->SBUF DMAs, one per partition-octet).

Numerics: end-to-end flips vs the fp32 reference come only from fp16
score-cast collisions (~0.1% of queries); measured rel err ~1e-4.
"""

import os, sys

for _p in (
    "/opt/trn_rl_repo",
    "/opt/pypackages",
    "/root/.axon_site/_ro/trn_rl_repo",
    "/root/.axon_site/_ro/pypackages",
):
    if os.path.isdir(_p) and _p not in sys.path:
        sys.path.insert(0, _p)

import numpy as np

import concourse.bass as bass
import concourse.bacc as bacc
import concourse.tile as tile
from concourse import masks, mybir

F32 = mybir.dt.float32
FP16 = mybir.dt.float16
I32 = mybir.dt.int32
U32 = mybir.dt.uint32
I16 = mybir.dt.int16
U16 = mybir.dt.uint16
A = mybir.AluOpType
AF = mybir.ActivationFunctionType
AX = mybir.AxisListType

B = 8
P, PC = 2048, 16            # queries, outer chunks of 128
NGT = 8192                  # gt vertices
VPR, VPAD = 2562, 2816      # pred vertices, padded to 128*22
TGT, TPR = 64, 22           # vertices per partition row (gt / pred)
NF, FCH = 16384, 128        # faces, face cols per partition
BIGC = 30.0                 # pad coordinate -> -d^2 ~ -2700, never wins
EPS = 1e-12
NEG = -60000.0              # fp16-safe -inf substitute

# group-of-4 tree: group j holds columns {j + STRIDE*k}
GSTR_GT = 2048              # 8192 / 4
GSTR_PR = 704               # 2816 / 4


def build_nc():
    nc = bacc.Bacc(None, target_bir_lowering=False)
    pp = nc.dram_tensor("pred_points", [P, 3], F32, kind="ExternalInput")
    pv = nc.dram_tensor("pred_vertices", [VPR, 3], F32, kind="ExternalInput")
    gv = nc.dram_tensor("gt_vertices", [NGT, 3], F32, kind="ExternalInput")
    gf = nc.dram_tensor("gt_faces32", [NF, 3], I32, kind="ExternalInput")
    gvp = nc.dram_tensor("gv_pad", [NGT, 64], F32)
    pvp = nc.dram_tensor("pv_pad", [VPAD, 64], F32)
    ndr = nc.dram_tensor("n_pad", [NGT, 64], F32)
    out = nc.dram_tensor("loss_sum", [1], F32, kind="ExternalOutput")

    from contextlib import ExitStack

    with tile.TileContext(nc) as tc, ExitStack() as ctx:
        _body(nc, tc, ctx, pp, pv, gv, gf, gvp, pvp, ndr, out)
    nc.compile()
    return nc


def _fold16(nc, dst, src, ncols):
    """dst[q, a*8+d] = src[16d+q, a]  (wrapped-index partition fold).

    src: [128, ncols]; dst: [*, 8*ncols] (rows 0:16 written).
    """
    for d in range(8):
        nc.sync.dma_start(
            out=dst[0:16, d : 8 * ncols : 8],
            in_=src[16 * d : 16 * (d + 1), 0:ncols],
        )


def _hi_lo(nc, work, src_f32, shape, tag):
    """Return (hi16, lo16) fp16 tiles: hi = fp16(x), lo = fp16(x - hi)."""
    hi = work.tile(shape, FP16, tag=f"{tag}_hi")
    nc.vector.tensor_copy(hi[:], src_f32[:])
    hif = work.tile(shape, F32, tag=f"{tag}_hif")
    nc.vector.tensor_copy(hif[:], hi[:])
    lof = work.tile(shape, F32, tag=f"{tag}_lof")
    nc.vector.tensor_tensor(out=lof[:], in0=src_f32[:], in1=hif[:], op=A.subtract)
    lo = work.tile(shape, FP16, tag=f"{tag}_lo")
    nc.vector.tensor_copy(lo[:], lof[:])
    return hi, lo


def _body(nc, tc, ctx, pp, pv, gv, gf, gvp, pvp, ndr, out_dram):
    sing = ctx.enter_context(tc.tile_pool(name="sing", bufs=1))
    work = ctx.enter_context(tc.tile_pool(name="work", bufs=2))
    stage = ctx.enter_context(tc.tile_pool(name="stage", bufs=1))

    # ================= setup: identity, loads =================
    with tc.tile_pool(name="mpsum", bufs=2, space=bass.MemorySpace.PSUM) as mpsum:
        ident0 = sing.tile([128, 128], F32)
        masks.make_identity(nc, ident0[:])
        ident = sing.tile([128, 128], F32)
        nc.vector.tensor_copy(ident[:], ident0[:])
        ident16 = sing.tile([128, 128], FP16)
        nc.vector.tensor_copy(ident16[:], ident0[:])

        # ---- queries: qRM2[p, i, c] = pp[128*i + p]  (query q = 128i+p)
        qRM2 = sing.tile([128, PC, 3], F32)
        nc.sync.dma_start(
            out=qRM2[:], in_=pp[:, :].rearrange("(i p) c -> p i c", p=128)
        )
        qsq3 = work.tile([128, PC, 3], F32, tag="qsq3")
        nc.vector.tensor_tensor(out=qsq3[:], in0=qRM2[:], in1=qRM2[:], op=A.mult)
        qsq = sing.tile([128, PC], F32)
        nc.vector.tensor_reduce(out=qsq[:], in_=qsq3[:], axis=AX.X, op=A.add)
        q_hi, q_lo = _hi_lo(nc, work, qRM2, [128, PC, 3], "q")

        # qCM [128, 7, 16]: rows 0-2 = 2*q_hi (c-major), 3-5 = 2*q_lo, 6 = -qsq
        qCM = work.tile([128, 7, PC], FP16, tag="qcm")
        nc.vector.tensor_scalar(
            out=qCM[:, 0:3, :],
            in0=q_hi[:].rearrange("p i c -> p c i"),
            scalar1=2.0, scalar2=None, op0=A.mult,
        )
        nc.vector.tensor_scalar(
            out=qCM[:, 3:6, :],
            in0=q_lo[:].rearrange("p i c -> p c i"),
            scalar1=2.0, scalar2=None, op0=A.mult,
        )
        nc.vector.tensor_scalar(
            out=qCM[:, 6, :], in0=qsq[:], scalar1=-1.0, scalar2=None, op0=A.mult
        )
        # transpose -> qT rows: [2qh(3), 2ql(3), -|q|^2] ; cols = query 128i+p
        qT = sing.tile([12, P], FP16)
        nc.vector.memset(qT[:], -1.0)  # rows 9,10 = -1 ; others overwritten
        qtp = mpsum.tile([112, 128], FP16, tag="tp16")
        nc.tensor.transpose(qtp[:], qCM[:].rearrange("p r i -> p (r i)"), ident16[:])
        qtsb = work.tile([112, 128], FP16, tag="qtsb")
        nc.vector.tensor_copy(qtsb[:], qtp[:])
        nc.sync.dma_start(
            out=qT[0:6, :].rearrange("r (i p) -> r i p", p=128),
            in_=qtsb[0:96, :],
        )
        nc.sync.dma_start(
            out=qT[11:12, :].rearrange("r (i p) -> r i p", p=128),
            in_=qtsb[96:112, :],
        )
        nc.sync.dma_start(out=qT[6:9, :], in_=qT[0:3, :])

        # ---- gt vertices: rRM[p, t, c] = gv[64p + t]; col n = vertex
        #      (n&127)*64 + (n>>7)
        rRM = sing.tile([128, TGT, 3], F32)
        nc.sync.dma_start(out=rRM[:], in_=gv[:, :].rearrange("(p t) c -> p t c", p=128))
        rsq3 = work.tile([128, TGT, 3], F32, tag="rsq3")
        nc.vector.tensor_tensor(out=rsq3[:], in0=rRM[:], in1=rRM[:], op=A.mult)
        rsq = sing.tile([128, TGT], F32)
        nc.vector.tensor_reduce(out=rsq[:], in_=rsq3[:], axis=AX.X, op=A.add)
        r_hi, r_lo = _hi_lo(nc, work, rRM, [128, TGT, 3], "r")
        rq_hi, rq_lo = _hi_lo(nc, work, rsq, [128, TGT], "rq")

        rT = sing.tile([12, NGT], FP16)
        nc.vector.memset(rT[:], 1.0)  # row 11 = 1
        # coords hi/lo: 2 halves of 32 t-cols each -> [96,128] transposes
        for src, rows in ((r_hi, 0), (r_lo, 6)):
            eng = nc.sync if rows == 0 else nc.scalar
            for h in range(2):
                cm = work.tile([128, 3, 32], FP16, tag="rcm")
                nc.vector.tensor_copy(
                    cm[:], src[:, 32 * h : 32 * (h + 1), :].rearrange("p t c -> p c t")
                )
                tp = mpsum.tile([96, 128], FP16, tag="tp16")
                nc.tensor.transpose(tp[:], cm[:].rearrange("p c t -> p (c t)"), ident16[:])
                sb = work.tile([96, 128], FP16, tag="rtsb")
                nc.vector.tensor_copy(sb[:], tp[:])
                eng.dma_start(
                    out=rT[rows : rows + 3, 4096 * h : 4096 * (h + 1)].rearrange(
                        "r (t p) -> r t p", p=128
                    ),
                    in_=sb[:],
                )
        # rows 3-5 duplicate r_hi
        nc.sync.dma_start(out=rT[3:6, :], in_=rT[0:3, :])
        # rsq hi/lo rows 9,10
        for src, row in ((rq_hi, 9), (rq_lo, 10)):
            tp = mpsum.tile([64, 128], FP16, tag="tp16")
            nc.tensor.transpose(tp[:], src[:], ident16[:])
            sb = work.tile([64, 128], FP16, tag="rtsb")
            nc.vector.tensor_copy(sb[:], tp[:])
            nc.scalar.dma_start(
                out=rT[row : row + 1, :].rearrange("r (t p) -> r t p", p=128),
                in_=sb[:],
            )

        # ---- pred vertices (padded to 2816): col n = vertex (n&127)*22+(n>>7)
        rRMp = sing.tile([128, TPR, 3], F32)
        nc.vector.memset(rRMp[:], BIGC)
        rRMp_f = rRMp[:].rearrange("p t c -> p (t c)")
        pv_f = pv[:, :].rearrange("v c -> (v c)")
        nc.sync.dma_start(
            out=rRMp_f[0:116, :], in_=pv_f[0 : 116 * 66].rearrange("(p a) -> p a", a=66)
        )
        nc.sync.dma_start(
            out=rRMp_f[116:117, 0:30], in_=pv_f[116 * 66 : 116 * 66 + 30].rearrange("(o a) -> o a", o=1)
        )
        psq3 = work.tile([128, TPR, 3], F32, tag="psq3")
        nc.vector.tensor_tensor(out=psq3[:], in0=rRMp[:], in1=rRMp[:], op=A.mult)
        psq = sing.tile([128, TPR], F32)
        nc.vector.tensor_reduce(out=psq[:], in_=psq3[:], axis=AX.X, op=A.add)
        p_hi, p_lo = _hi_lo(nc, work, rRMp, [128, TPR, 3], "p")
        pq_hi, pq_lo = _hi_lo(nc, work, psq, [128, TPR], "pq")

        rTp = sing.tile([12, VPAD], FP16)
        nc.vector.memset(rTp[:], 1.0)
        for src, rows in ((p_hi, 0), (p_lo, 6)):
            cm = work.tile([128, 3, TPR], FP16, tag="pcm")
            nc.vector.tensor_copy(cm[:], src[:].rearrange("p t c -> p c t"))
            tp = mpsum.tile([66, 128], FP16, tag="tp16")
            nc.tensor.transpose(tp[:], cm[:].rearrange("p c t -> p (c t)"), ident16[:])
            sb = work.tile([66, 128], FP16, tag="ptsb")
            nc.vector.tensor_copy(sb[:], tp[:])
            nc.scalar.dma_start(
                out=rTp[rows : rows + 3, :].rearrange("r (t p) -> r t p", p=128),
                in_=sb[:],
            )
        nc.scalar.dma_start(out=rTp[3:6, :], in_=rTp[0:3, :])
        for src, row in ((pq_hi, 9), (pq_lo, 10)):
            tp = mpsum.tile([TPR, 128], FP16, tag="tp16")
            nc.tensor.transpose(tp[:], src[:], ident16[:])
            sb = work.tile([TPR, 128], FP16, tag="ptsb")
            nc.vector.tensor_copy(sb[:], tp[:])
            nc.scalar.dma_start(
                out=rTp[row : row + 1, :].rearrange("r (t p) -> r t p", p=128),
                in_=sb[:],
            )

        # ---- padded DRAM arrays: gv_pad/pv_pad rows [x,y,z,rsq,0...]
        gstg = stage.tile([128, TGT, 64], F32, tag="gstg")
        nc.gpsimd.memset(gstg[:], 0.0)
        nc.vector.tensor_copy(gstg[:, :, 0:3], rRM[:])
        nc.vector.tensor_copy(gstg[:, :, 3], rsq[:])
        nc.sync.dma_start(
            out=gvp[:, :].rearrange("(p t) e -> p (t e)", p=128),
            in_=gstg[:].rearrange("p t e -> p (t e)"),
        )
        pstg = stage.tile([128, TPR, 64], F32, tag="pstg")
        nc.gpsimd.memset(pstg[:], 0.0)
        nc.vector.tensor_copy(pstg[:, :, 0:3], rRMp[:])
        nc.vector.tensor_copy(pstg[:, :, 3], psq[:])
        nc.sync.dma_start(
            out=pvp[:, :].rearrange("(p t) e -> p (t e)", p=128),
            in_=pstg[:].rearrange("p t e -> p (t e)"),
        )
        # zero n_pad
        zt = stage.tile([128, 4096], F32, tag="gstg")
        nc.gpsimd.memset(zt[:], 0.0)
        nc.sync.dma_start(
            out=ndr[:, :].rearrange("(p t) e -> p (t e)", p=128), in_=zt[:]
        )

        # ---- faces + wrapped corner-index tiles (2 halves of 64 cols)
        faces = sing.tile([128, FCH, 3], I32)
        nc.sync.dma_start(
            out=faces[:], in_=gf[:, :].rearrange("(p ch) w -> p ch w", p=128)
        )
        Wf = []
        for h in range(2):
            F2 = sing.tile([128, 192], I16, tag=f"f2_{h}")
            nc.vector.tensor_copy(
                F2[:].rearrange("p (c t) -> p c t", c=3),
                faces[:, 64 * h : 64 * (h + 1), :].rearrange("p ch c -> p c ch"),
            )
            W = sing.tile([128, 1536], I16, tag=f"wf_{h}")
            nc.vector.memset(W[:], 0)
            _fold16(nc, W, F2, 192)
            Wf.append(W)

        # iota ramps for candidate-id math
        iot128_i = sing.tile([128, 128], I32)
        nc.gpsimd.iota(iot128_i[:], pattern=[[1, 128]], base=0, channel_multiplier=0)
        # kramp4[r, c] = c // 32  (batch idx tiles: col = 8di + d + 32k)
        kramp = sing.tile([128, 128], I32)
        nc.vector.tensor_scalar(
            out=kramp[:], in0=iot128_i[:], scalar1=5, scalar2=None,
            op0=A.logical_shift_right,
        )
        # iota8k[p, i, k] = k  for rescore select
        io4_i = sing.tile([128, PC, 4], I32)
        nc.gpsimd.iota(io4_i[:], pattern=[[0, PC], [1, 4]], base=0, channel_multiplier=0)
        io4 = sing.tile([128, PC, 4], F32)
        nc.vector.tensor_copy(io4[:], io4_i[:])

    # ================= normals: gather corners, cross, scatter ============
    # (instructions emitted up-front; tile deps let them overlap the search)
    vg_pool = ctx.enter_context(tc.tile_pool(name="vg", bufs=1))

    def normals_half(h):
        Vg = vg_pool.tile([128, 192, 64], F32, tag="vg")
        for g in range(6):
            nc.gpsimd.dma_gather(
                out_ap=Vg[:, 32 * g : 32 * (g + 1), :], in_ap=gvp[:, :],
                idxs_ap=Wf[h][:, 256 * g : 256 * (g + 1)],
                num_idxs=4096, num_idxs_reg=4096, elem_size=64,
            )
        # cross products: blocks of 64 cols per corner
        eA = work.tile([128, 64, 3], F32, tag="eA")
        eB = work.tile([128, 64, 3], F32, tag="eB")
        nc.vector.tensor_tensor(
            out=eA[:], in0=Vg[:, 64:128, 0:3], in1=Vg[:, 0:64, 0:3], op=A.subtract
        )
        nc.vector.tensor_tensor(
            out=eB[:], in0=Vg[:, 128:192, 0:3], in1=Vg[:, 0:64, 0:3], op=A.subtract
        )
        fn = work.tile([128, 64, 3], F32, tag="fn")
        for d in range(3):
            u, v = (d + 1) % 3, (d + 2) % 3
            t1 = work.tile([128, 64], F32, tag="cr1")
            t2 = work.tile([128, 64], F32, tag="cr2")
            nc.vector.tensor_tensor(out=t1[:], in0=eA[:, :, u], in1=eB[:, :, v], op=A.mult)
            nc.vector.tensor_tensor(out=t2[:], in0=eA[:, :, v], in1=eB[:, :, u], op=A.mult)
            nc.vector.tensor_tensor(out=fn[:, :, d], in0=t1[:], in1=t2[:], op=A.subtract)
        for c in range(3):
            nc.vector.tensor_copy(Vg[:, 64 * c : 64 * c + 64, 0:3], fn[:])
            # clear the rsq slot so n_pad col 3 stays clean
            nc.vector.memset(Vg[:, 64 * c : 64 * c + 64, 3], 0.0)
        for g in range(6):
            nc.gpsimd.dma_scatter_add(
                ndr[:, :], Vg[:, 32 * g : 32 * (g + 1), :],
                Wf[h][:, 256 * g : 256 * (g + 1)],
                num_idxs=4096, num_idxs_reg=4096, elem_size=64,
            )

    # ================= searches =================
    sc_pool = ctx.enter_context(tc.tile_pool(name="scores", bufs=3))

    g_gt = sing.tile([128, PC], I32)   # winning group id per (p, i)
    g_pr = sing.tile([128, PC], I32)
    v_gt = sing.tile([128, PC], I32)   # final vertex ids
    v_pr = sing.tile([128, PC], I32)

    with tc.tile_pool(name="spsum", bufs=2, space=bass.MemorySpace.PSUM) as spsum:

        def search_outer(i, rT_t, ncols, g_out):
            """One outer chunk: matmuls -> evac fp16 -> tree -> MaxIndex."""
            s16 = sc_pool.tile([128, 8192], FP16, tag="s16")
            qs = qT[:, 128 * i : 128 * (i + 1)]
            for m0 in range(0, ncols, 2048):
                mw = min(2048, ncols - m0)
                ps = spsum.tile([128, 2048], F32, tag="ps")
                for c0 in range(0, mw, 512):
                    cw = min(512, mw - c0)
                    nc.tensor.matmul(
                        ps[:, c0 : c0 + cw], qs, rT_t[:, m0 + c0 : m0 + c0 + cw],
                        start=True, stop=True,
                    )
                nc.scalar.copy(s16[:, m0 : m0 + mw], ps[:, 0:mw])
            qw = ncols // 4
            # pair adjacent quarters first so level-1a only needs megas 0-1
            nc.vector.tensor_tensor(
                out=s16[:, 0:qw], in0=s16[:, 0:qw], in1=s16[:, qw : 2 * qw], op=A.max
            )
            nc.vector.tensor_tensor(
                out=s16[:, 2 * qw : 3 * qw], in0=s16[:, 2 * qw : 3 * qw],
                in1=s16[:, 3 * qw : 4 * qw], op=A.max,
            )
            nc.vector.tensor_tensor(
                out=s16[:, 0:qw], in0=s16[:, 0:qw], in1=s16[:, 2 * qw : 3 * qw],
                op=A.max,
            )
            rm = work.tile([128, 1], F32, tag="rm")
            nc.vector.tensor_scalar(
                out=s16[:, 0:qw], in0=s16[:, 0:qw], scalar1=NEG, scalar2=None,
                op0=A.max, op1=A.max, accum_out=rm[:],
            )
            mx8 = work.tile([128, 8], FP16, tag="mx8")
            nc.vector.tensor_copy(mx8[:], rm[:].to_broadcast([128, 8]))
            ix8 = work.tile([128, 8], U32, tag="ix8")
            nc.vector.max_index(ix8[:], mx8[:], s16[:, 0:qw])
            nc.vector.tensor_copy(g_out[:, i : i + 1], ix8[:, 0:1])

        vg_pool = tc.alloc_tile_pool(name="vg", bufs=1)

        def normals_half(h):
            Vg = vg_pool.tile([128, 192, 64], F32, tag="vg")
            for g in range(6):
                nc.gpsimd.dma_gather(
                    out_ap=Vg[:, 32 * g : 32 * (g + 1), :], in_ap=gvp[:, :],
                    idxs_ap=Wf[h][:, 256 * g : 256 * (g + 1)],
                    num_idxs=4096, num_idxs_reg=4096, elem_size=64,
                )
            eA = work.tile([128, 64, 3], F32, tag="eA")
            eB = work.tile([128, 64, 3], F32, tag="eB")
            nc.vector.tensor_tensor(
                out=eA[:], in0=Vg[:, 64:128, 0:3], in1=Vg[:, 0:64, 0:3], op=A.subtract
            )
            nc.vector.tensor_tensor(
                out=eB[:], in0=Vg[:, 128:192, 0:3], in1=Vg[:, 0:64, 0:3], op=A.subtract
            )
            fn = work.tile([128, 64, 3], F32, tag="fn")
            for d in range(3):
                u, v = (d + 1) % 3, (d + 2) % 3
                t1 = work.tile([128, 64], F32, tag="cr1")
                t2 = work.tile([128, 64], F32, tag="cr2")
                nc.vector.tensor_tensor(out=t1[:], in0=eA[:, :, u], in1=eB[:, :, v], op=A.mult)
                nc.vector.tensor_tensor(out=t2[:], in0=eA[:, :, v], in1=eB[:, :, u], op=A.mult)
                nc.vector.tensor_tensor(out=fn[:, :, d], in0=t1[:], in1=t2[:], op=A.subtract)
            for c in range(3):
                nc.vector.tensor_copy(Vg[:, 64 * c : 64 * c + 64, 0:3], fn[:])
                nc.vector.memset(Vg[:, 64 * c : 64 * c + 64, 3], 0.0)
            for g in range(6):
                nc.gpsimd.dma_scatter_add(
                    ndr[:, :], Vg[:, 32 * g : 32 * (g + 1), :],
                    Wf[h][:, 256 * g : 256 * (g + 1)],
                    num_idxs=4096, num_idxs_reg=4096, elem_size=64,
                )

        def rescore_search(rsc, g_all, v_out, stride, tmul, src_d, nrows, tag):
            """Exact fp32 rescore of the 4 candidates per query for one search."""
            colk = sing.tile([128, PC, 4], I32, tag=f"colk_{tag}")
            nc.vector.tensor_scalar(
                out=colk[:], in0=io4_i[:], scalar1=stride, scalar2=None, op0=A.mult
            )
            nc.vector.tensor_tensor(
                out=colk[:], in0=colk[:],
                in1=g_all[:].rearrange("p (i o) -> p i o", o=1).to_broadcast([128, PC, 4]),
                op=A.add,
            )
            vall = sing.tile([128, PC, 4], I32, tag=f"vall_{tag}")
            nc.vector.tensor_scalar(
                out=vall[:], in0=colk[:], scalar1=127, scalar2=None, op0=A.bitwise_and
            )
            nc.vector.tensor_scalar(
                out=vall[:], in0=vall[:], scalar1=tmul, scalar2=None, op0=A.mult
            )
            hi = sing.tile([128, PC, 4], I32, tag=f"hi_{tag}")
            nc.vector.tensor_scalar(
                out=hi[:], in0=colk[:], scalar1=7, scalar2=None, op0=A.logical_shift_right
            )
            nc.vector.tensor_tensor(out=vall[:], in0=vall[:], in1=hi[:], op=A.add)
            # k-major int16 copy so each fold DMA is a 1-dim run
            v16 = sing.tile([128, 4, PC], I16, tag=f"v16_{tag}")
            nc.vector.tensor_copy(v16[:], vall[:].rearrange("p i k -> p k i"))
            Ws = sing.tile([128, 512], I16, tag=f"ws_{tag}")
            nc.vector.memset(Ws[:], 0)
            for d in range(8):
                nc.sync.dma_start(
                    out=Ws[0:16, d:512:8],
                    in_=v16[16 * d : 16 * (d + 1), :, :],
                )
            Vc = rsc.tile([128, 64, 64], F32, tag="vc")
            for g in range(2):
                nc.gpsimd.dma_gather(
                    out_ap=Vc[:, 32 * g : 32 * (g + 1), :], in_ap=src_d[:, :],
                    idxs_ap=Ws[:, 256 * g : 256 * (g + 1)],
                    num_idxs=4096, num_idxs_reg=4096, elem_size=64,
                )
            # Vc[p, 16k + i, :]; score = 2 q.v - |v|^2
            prod = rsc.tile([128, 4, PC, 3], F32, tag="prod")
            nc.vector.tensor_tensor(
                out=prod[:],
                in0=Vc[:, :, 0:3].rearrange("p (k i) e -> p k i e", k=4),
                in1=qRM2[:].rearrange("p i (o e) -> p o i e", o=1).to_broadcast(
                    [128, 4, PC, 3]
                ),
                op=A.mult,
            )
            dot = rsc.tile([128, 4, PC], F32, tag="dotc")
            nc.vector.tensor_reduce(out=dot[:], in_=prod[:], axis=AX.X, op=A.add)
            sc = rsc.tile([128, 4, PC], F32, tag="scc")
            nc.vector.tensor_scalar(
                out=sc[:], in0=dot[:], scalar1=2.0, scalar2=None, op0=A.mult
            )
            nc.vector.tensor_tensor(
                out=sc[:], in0=sc[:],
                in1=Vc[:, :, 3].rearrange("p (k i) -> p k i", k=4), op=A.subtract,
            )
            scd = rsc.tile([128, PC, 4], F32, tag="scd")
            nc.vector.tensor_copy(scd[:], sc[:].rearrange("p k i -> p i k"))
            mbest = rsc.tile([128, PC], F32, tag="mb")
            nc.vector.tensor_reduce(out=mbest[:], in_=scd[:], axis=AX.X, op=A.max)
            eqm = rsc.tile([128, PC, 4], F32, tag="eq")
            nc.vector.tensor_tensor(
                out=eqm[:], in0=scd[:],
                in1=mbest[:].rearrange("p (i o) -> p i o", o=1).to_broadcast([128, PC, 4]),
                op=A.is_equal,
            )
            nc.vector.tensor_tensor(out=eqm[:], in0=eqm[:], in1=io4[:], op=A.mult)
            kbest = rsc.tile([128, PC], F32, tag="kb")
            nc.vector.tensor_reduce(out=kbest[:], in_=eqm[:], axis=AX.X, op=A.add)
            kb_i = rsc.tile([128, PC], I32, tag="kbi")
            nc.vector.tensor_copy(kb_i[:], kbest[:])
            col = rsc.tile([128, PC], I32, tag="colf")
            nc.vector.tensor_scalar(
                out=col[:], in0=kb_i[:], scalar1=stride, scalar2=None, op0=A.mult
            )
            nc.vector.tensor_tensor(out=col[:], in0=col[:], in1=g_all[:], op=A.add)
            vlo = rsc.tile([128, PC], I32, tag="vlo")
            nc.vector.tensor_scalar(
                out=vlo[:], in0=col[:], scalar1=127, scalar2=None, op0=A.bitwise_and
            )
            nc.vector.tensor_scalar(
                out=vlo[:], in0=vlo[:], scalar1=tmul, scalar2=None, op0=A.mult
            )
            nc.vector.tensor_scalar(
                out=col[:], in0=col[:], scalar1=7, scalar2=None,
                op0=A.logical_shift_right,
            )
            nc.vector.tensor_tensor(out=col[:], in0=col[:], in1=vlo[:], op=A.add)
            nc.vector.tensor_scalar(
                out=col[:], in0=col[:], scalar1=0, scalar2=nrows - 1,
                op0=A.max, op1=A.min,
            )
            nc.vector.tensor_copy(v_out[:], col[:])

        normals_half(0)
        for i in range(PC):
            search_outer(i, rT, NGT, g_gt)
            if i == 1:
                normals_half(1)
            if i == 2:
                vg_pool.release()   # free 48KB for the rescore pool

        rsc = ctx.enter_context(tc.tile_pool(name="rsc", bufs=1))

        def gather_by_idx(v_all, src_d, tag):
            vi16 = sing.tile([128, PC], I16, tag=f"vi16_{tag}")
            nc.vector.tensor_copy(vi16[:], v_all[:])
            W = sing.tile([128, 128], I16, tag=f"wg_{tag}")
            nc.vector.memset(W[:], 0)
            _fold16(nc, W, vi16, PC)
            dst = sing.tile([128, PC, 64], F32, tag=f"gth_{tag}")
            nc.gpsimd.dma_gather(
                out_ap=dst[:], in_ap=src_d[:, :], idxs_ap=W[:],
                num_idxs=2048, num_idxs_reg=2048, elem_size=64,
            )
            return dst

        rescore_search(rsc, g_gt, v_gt, GSTR_GT, TGT, gvp, NGT, "g")
        # normal-side epilogue overlaps the pred searches
        nrm = gather_by_idx(v_gt, ndr, "n")
        nn_t = work.tile([128, PC, 3], F32, tag="en")
        nc.vector.tensor_tensor(
            out=nn_t[:], in0=nrm[:, :, 0:3], in1=nrm[:, :, 0:3], op=A.mult
        )
        nn = sing.tile([128, PC], F32)
        nc.vector.tensor_reduce(out=nn[:], in_=nn_t[:], axis=AX.X, op=A.add)
        nlen = sing.tile([128, PC], F32)
        nc.scalar.activation(nlen[:], nn[:], AF.Sqrt)
        nc.vector.tensor_scalar(out=nlen[:], in0=nlen[:], scalar1=EPS, scalar2=None, op0=A.max)

        for i in range(PC):
            search_outer(i, rTp, VPAD, g_pr)
        rescore_search(rsc, g_pr, v_pr, GSTR_PR, TPR, pvp, VPAD, "p")


    # ================= epilogue =================
    with tc.tile_pool(name="epsum", bufs=1, space=bass.MemorySpace.PSUM) as epsum:
        vtx = gather_by_idx(v_pr, pvp, "v")

        e = sing.tile([128, PC, 3], F32)
        nc.vector.tensor_tensor(out=e[:], in0=qRM2[:], in1=vtx[:, :, 0:3], op=A.subtract)
        tmp3 = work.tile([128, PC, 3], F32, tag="en")
        nc.vector.tensor_tensor(out=tmp3[:], in0=e[:], in1=nrm[:, :, 0:3], op=A.mult)
        dot = sing.tile([128, PC], F32)
        nc.vector.tensor_reduce(out=dot[:], in_=tmp3[:], axis=AX.X, op=A.add)
        ee_t = work.tile([128, PC, 3], F32, tag="en")
        nc.vector.tensor_tensor(out=ee_t[:], in0=e[:], in1=e[:], op=A.mult)
        ee = sing.tile([128, PC], F32)
        nc.vector.tensor_reduce(out=ee[:], in_=ee_t[:], axis=AX.X, op=A.add)

        elen = sing.tile([128, PC], F32)
        nc.scalar.activation(elen[:], ee[:], AF.Sqrt)
        nc.vector.tensor_scalar(out=elen[:], in0=elen[:], scalar1=EPS, scalar2=None, op0=A.max)
        den = sing.tile([128, PC], F32)
        nc.vector.tensor_tensor(out=den[:], in0=elen[:], in1=nlen[:], op=A.mult)
        rden = sing.tile([128, PC], F32)
        nc.vector.reciprocal(rden[:], den[:])
        res = sing.tile([128, PC], F32)
        nc.vector.tensor_tensor(out=res[:], in0=dot[:], in1=rden[:], op=A.mult)
        partial = sing.tile([128, 1], F32)
        nc.vector.tensor_reduce(
            out=partial[:], in_=res[:], axis=AX.X, op=A.add, apply_absolute_value=True
        )
        ones = sing.tile([128, 1], F32)
        nc.vector.memset(ones[:], 1.0)
        fps = epsum.tile([1, 1], F32, tag="fin")
        nc.tensor.matmul(fps[:], ones[:], partial[:], start=True, stop=True)
        osb = sing.tile([1, 1], F32)
        nc.scalar.copy(osb[:], fps[:])
        nc.sync.dma_start(out=out_dram[:], in_=osb[:])


_NC_CACHE = None


def _get_nc():
    global _NC_CACHE
    if _NC_CACHE is None:
        _NC_CACHE = build_nc()
    return _NC_CACHE


def make_in_maps(pred_points, pred_vertices, gt_vertices, gt_faces):
    nb = pred_points.shape[0]
    faces32 = np.asarray(gt_faces).astype(np.int32, copy=False)
    return [
        dict(
            pred_points=np.ascontiguousarray(pred_points[b], dtype=np.float32),
            pred_vertices=np.ascontiguousarray(pred_vertices[b], dtype=np.float32),
            gt_vertices=np.ascontiguousarray(gt_vertices[b], dtype=np.float32),
            gt_faces32=np.ascontiguousarray(faces32[b]),
        )
        for b in range(nb)
    ]


def kernel(pred_points, pred_vertices, gt_vertices, gt_faces):
    from concourse.bass_utils import run_bass_kernel_spmd

    nb = pred_points.shape[0]
    nc = _get_nc()
    in_maps = make_in_maps(pred_points, pred_vertices, gt_vertices, gt_faces)
    res = run_bass_kernel_spmd(nc, in_maps, list(range(nb)))
    total = sum(float(res.results[i]["loss_sum"][0]) for i in range(nb))
    return np.array(total / (nb * P), dtype=np.float32)


if __name__ == "__main__":
    nc = build_nc()
    print("built ok")
